# revision 6
# baseline (speedup 1.0000x reference)
"""Masked attention on 8 TRN2 NeuronCores — pure data-parallel over batch.

Full inputs:  q,k,v (16,2048,128) f32, mask (16,2048,2048) bool.
Output:       (16,2048,128) f32.

Per core (2 batches), per 512-q x 128-k score tile:

  scores + mask land in PSUM via TWO fp8 DoubleRow matmuls (each contracts
  2x128 rows at 0.5 cyc/out-row):
    pass1: k_hi . q_hi  +  (-240*I) . mask01     (mask folded into the matmul)
    pass2: k_lo . q_hi  +  k_hi . q_lo           (hi-lo fp8 ~ bf16 precision)
  q/k are split host-side into e4m3 hi/lo pairs; the q_lo*k_lo term is
  dropped (second order).  The two operand pairs of each DoubleRow matmul
  are addressed with strided chunk APs into one big SBUF tile, so no data
  is duplicated.

  exp is split across two engines to break the ACT throughput wall:
    ~5/8 of tiles: ScalarE  attn = exp(scale*psum)            -> bf16
    ~3/8 of tiles: VectorE  attn = bitcast_bf16(int16(A*psum+B))
  (Schraudolph-style exponential: the int16 bits of A*x+B reinterpreted as
  bf16 approximate exp(x*scale) to ~1.5% — measured end-to-end rel err
  1.3e-2 vs the 2e-2 gate.)  Masked entries were pushed down by the -240
  mask term so both paths produce ~0.

  AV accumulates [128q, 129] per q-subblock in bf16 with a ones-column in
  va giving the softmax denominator; normalize = reciprocal + scale on
  DVE/ACT into a staging tile; one DMA per 1024-q group stores the output
  in natural [q, d] layout.
"""

import numpy as np
import ml_dtypes

B, S, D = 16, 2048, 128
N_CORES = 8
BPC = B // N_CORES   # batches per core
P = 128              # partitions
QW = 512             # q-tile width (one PSUM bank of f32)
KB = S // P          # k-blocks per batch (16)
NQT = S // QW        # q-tiles per batch (4)
NQG = NQT // 2       # q-groups: 2 q-tiles per group
NPAIR = KB // 2      # k-block pairs per group pass (8)

SCALE = float(1.0 / np.sqrt(np.float32(128.0)))
A2 = float(128.0 * np.log2(np.e) * SCALE)   # Schraudolph slope
B_U = 16249.0                               # Schraudolph bias (calibrated)
C_MASK = -240.0                             # mask diag coefficient (e4m3 max)
DVE_UNITS = (1, 4, 6)                       # of every 8 (g,t,qh) units

_NC = None
LAST_RESULT = None  # BassKernelResults of the most recent run (for profiling)


def _build_nc(bpc=BPC, s=S):
    import concourse.bacc as bacc
    import concourse.tile as tile
    from concourse import mybir

    BPC_, S_ = bpc, s
    KB_ = S_ // P
    NQT_ = S_ // QW
    NQG_ = NQT_ // 2
    NPAIR_ = KB_ // 2
    PD = 3            # AV matmuls pipelined this many pairs behind exp
    DA = D + 1

    bf16 = mybir.dt.bfloat16
    f32 = mybir.dt.float32
    fp8 = mybir.dt.float8e4
    i16 = mybir.dt.int16
    DR = mybir.MatmulPerfMode.DoubleRow
    MUL = mybir.AluOpType.mult
    ADD = mybir.AluOpType.add

    nc = bacc.Bacc()
    # all [d, s]-transposed fp8 hi/lo halves of q and k
    qhT = nc.declare_dram_parameter("qhT", [BPC_, P, S_], fp8, isOutput=False)
    qlT = nc.declare_dram_parameter("qlT", [BPC_, P, S_], fp8, isOutput=False)
    khT = nc.declare_dram_parameter("khT", [BPC_, P, S_], fp8, isOutput=False)
    klT = nc.declare_dram_parameter("klT", [BPC_, P, S_], fp8, isOutput=False)
    # mask01[b, k, q] = 1.0 where masked else 0.0 (host-transposed)
    m8 = nc.declare_dram_parameter("m8", [BPC_, S_, S_], fp8, isOutput=False)
    # -240 * I
    diag = nc.declare_dram_parameter("diag", [P, P], fp8, isOutput=False)
    # va host-packed as [p, kb*(D+1)]: row p holds v[kb*128+p, :]+[1] per kb
    va = nc.declare_dram_parameter(
        "va", [BPC_, P, KB_ * DA], bf16, isOutput=False
    )
    out = nc.declare_dram_parameter("out", [BPC_, S_, D], bf16, isOutput=True)

    with tile.TileContext(nc) as tc:
        with (
            tc.tile_pool(name="km", bufs=2) as kmp,
            tc.tile_pool(name="qm", bufs=2) as qmp,
            tc.tile_pool(name="vp", bufs=2) as vp,
            tc.tile_pool(name="attn", bufs=10) as attnp,
            tc.tile_pool(name="stg", bufs=3) as stgp,
            tc.tile_pool(name="rp", bufs=8) as rp,
            tc.tile_pool(name="const", bufs=1) as constp,
            tc.tile_pool(name="spsum", bufs=2, space="PSUM") as spsum,
            tc.tile_pool(name="avpsum", bufs=4, space="PSUM") as avpsum,
        ):
            # dummy exp so the activation-table load overlaps initial DMAs
            warm = constp.tile([P, 1], f32)
            nc.vector.memset(warm[:], 0.0)
            nc.scalar.activation(
                warm[:], warm[:], mybir.ActivationFunctionType.Exp
            )
            warm8 = constp.tile([P, P], fp8)
            nc.vector.memset(warm8[:], 0.0)
            # PE warm-up burst: ramps the p-state clock before real matmuls
            wp = avpsum.tile([P, P], f32, name="warm_mm", tag="av")
            for _ in range(8):
                nc.tensor.matmul(
                    wp[:], lhsT=warm8[:], rhs=warm8[:], start=True, stop=True
                )
            for b in range(BPC_):
                # KM chunks(128): [0..15]=klT [16..31]=khT [32]=diag
                km_s = kmp.tile([P, 33 * P], fp8)
                # QM chunks(512): [0..31]=masks kb0-7, [32..35]=qhT,
                # [36..39]=qlT, [40..71]=masks kb8-15.  q sits in the middle
                # so every (q-chunk, mask-chunk) stride stays within the
                # 16-bit step_elem ISA field (|step| <= 16384 elements).
                qm_s = qmp.tile([P, 72 * QW], fp8)
                va_s = vp.tile([P, KB_, DA], bf16)

                def _mchunk(kb):
                    return 4 * kb if kb < 8 else 40 + 4 * (kb - 8)

                def _ldm(kb):
                    c0 = _mchunk(kb)
                    nc.sync.dma_start(
                        out=qm_s[:, c0 * QW : (c0 + 4) * QW],
                        in_=m8[b, kb * P : (kb + 1) * P, :],
                    )

                # issue order tuned so early consumers never wait long
                nc.sync.dma_start(
                    out=km_s[:, 32 * P : 33 * P], in_=diag[:, :]
                )
                nc.sync.dma_start(
                    out=km_s[:, 16 * P : 32 * P], in_=khT[b, :, :]
                )
                nc.sync.dma_start(
                    out=qm_s[:, 32 * QW : 36 * QW], in_=qhT[b, :, :]
                )
                _ldm(0)
                _ldm(1)
                nc.sync.dma_start(
                    out=qm_s[:, 36 * QW : 40 * QW], in_=qlT[b, :, :]
                )
                nc.sync.dma_start(out=km_s[:, 0 : 16 * P], in_=klT[b, :, :])
                _ldm(2)
                _ldm(3)
                nc.sync.dma_start(
                    out=va_s[:, :, :],
                    in_=va[b, :, :].rearrange("p (kb d) -> p kb d", d=DA),
                )
                for kb in range(4, KB_):
                    _ldm(kb)

                km3 = km_s[:].rearrange("p (c n) -> p c n", n=P)
                qm3 = qm_s[:].rearrange("p (c n) -> p c n", n=QW)

                for g in range(NQG_):
                    av_tri = [
                        avpsum.tile([P, 3, DA], f32, name="av_tri", tag="av")
                        for _ in range(3)
                    ]
                    av_ps = [av_tri[sl // 3][:, sl % 3, :] for sl in range(8)]
                    stage = stgp.tile([P, 8, P], bf16)
                    attn_tiles = [[None, None] for _ in range(NPAIR_)]
                    for t in range(NPAIR_ + PD):
                        if t < NPAIR_:
                            for qh in range(2):
                                qx = g * 2 + qh
                                s_ps = spsum.tile([P, 2, QW], f32)
                                for h in range(2):
                                    kb = 2 * t + h
                                    # pass1: kh.qh + diag.mask
                                    qa = 32 + qx
                                    mc = (
                                        4 * kb if kb < 8 else 40 + 4 * (kb - 8)
                                    ) + qx
                                    sr = mc - qa  # negative for kb<8
                                    stop = mc + (1 if sr > 0 else -1)
                                    if stop < 0:
                                        stop = None
                                    nc.tensor.matmul(
                                        s_ps[:, h, :],
                                        lhsT=km3[
                                            :, 16 + kb : 33 : 16 - kb, :
                                        ],
                                        rhs=qm3[:, qa : stop : sr, :],
                                        start=True,
                                        stop=False,
                                        perf_mode=DR,
                                    )
                                    # pass2: kl.qh + kh.ql
                                    nc.tensor.matmul(
                                        s_ps[:, h, :],
                                        lhsT=km3[:, kb : kb + 17 : 16, :],
                                        rhs=qm3[:, qa : qa + 5 : 4, :],
                                        start=False,
                                        stop=True,
                                        perf_mode=DR,
                                    )
                                unit = (g * NPAIR_ + t) * 2 + qh
                                at = attnp.tile([P, 2, QW], bf16)
                                if (unit % 8) in DVE_UNITS:
                                    nc.vector.tensor_scalar(
                                        at[:, :, :].bitcast(i16),
                                        s_ps[:, :, :],
                                        A2,
                                        B_U,
                                        MUL,
                                        ADD,
                                    )
                                else:
                                    nc.scalar.activation(
                                        at[:, :, :],
                                        s_ps[:, :, :],
                                        mybir.ActivationFunctionType.Exp,
                                        scale=SCALE,
                                    )
                                attn_tiles[t][qh] = at
                        if t >= PD:
                            tp = t - PD
                            for qh in range(2):
                                ats = attn_tiles[tp][qh]
                                for h in range(2):
                                    kb = 2 * tp + h
                                    for qi in range(4):
                                        sl = qh * 4 + qi
                                        nc.tensor.matmul(
                                            av_ps[sl][:, :],
                                            lhsT=ats[
                                                :, h, qi * P : (qi + 1) * P
                                            ],
                                            rhs=va_s[:, kb, :],
                                            start=(kb == 0 and sl % 3 == 0),
                                            stop=(
                                                kb == KB_ - 1
                                                and (sl % 3 == 2 or sl == 7)
                                            ),
                                        )
                    # normalize into the staging tile, alternating engines
                    for sl in range(8):
                        recip = rp.tile([P, 1], f32)
                        nc.vector.reciprocal(recip[:], av_ps[sl][:, D : D + 1])
                        if sl % 2 == 1:
                            nc.scalar.activation(
                                stage[:, sl, :],
                                av_ps[sl][:, 0:D],
                                mybir.ActivationFunctionType.Copy,
                                scale=recip[:],
                            )
                        else:
                            nc.vector.tensor_scalar_mul(
                                stage[:, sl, :], av_ps[sl][:, 0:D], recip[:]
                            )
                    out_ap = out[
                        b, g * 2 * QW : (g + 1) * 2 * QW, :
                    ].rearrange("(sl q) d -> q sl d", sl=8)
                    nc.sync.dma_start(out=out_ap, in_=stage[:, :, :])
    nc.compile()
    return nc


def kernel(q, k, v, mask, _trace=False, _trace_kwargs=None):
    global _NC, LAST_RESULT
    from concourse.bass_utils import run_bass_kernel_spmd

    if _NC is None:
        _NC = _build_nc()

    bf = ml_dtypes.bfloat16
    e4 = ml_dtypes.float8_e4m3

    qT = np.ascontiguousarray(np.asarray(q, np.float32).transpose(0, 2, 1))
    kT = np.ascontiguousarray(np.asarray(k, np.float32).transpose(0, 2, 1))
    qh8 = qT.astype(e4)
    ql8 = (qT - qh8.astype(np.float32)).astype(e4)
    kh8 = kT.astype(e4)
    kl8 = (kT - kh8.astype(np.float32)).astype(e4)
    m8_full = np.ascontiguousarray(
        np.asarray(mask, bool).transpose(0, 2, 1)
    ).astype(e4)
    diag = (C_MASK * np.eye(P, dtype=np.float32)).astype(e4)
    ones = np.ones((B, S, 1), dtype=np.float32)
    va_full = (
        np.concatenate([np.asarray(v, np.float32), ones], axis=2)
        .reshape(B, S // P, P, D + 1)
        .transpose(0, 2, 1, 3)
        .reshape(B, P, (S // P) * (D + 1))
        .astype(bf)
    )

    in_maps = []
    for c in range(N_CORES):
        lo, hi = c * BPC, (c + 1) * BPC
        in_maps.append(
            {
                "qhT": qh8[lo:hi],
                "qlT": ql8[lo:hi],
                "khT": kh8[lo:hi],
                "klT": kl8[lo:hi],
                "m8": m8_full[lo:hi],
                "diag": diag,
                "va": va_full[lo:hi],
            }
        )

    kw = {}
    if _trace:
        kw["trace"] = True
        if _trace_kwargs:
            kw.update(_trace_kwargs)
    LAST_RESULT = run_bass_kernel_spmd(_NC, in_maps, list(range(N_CORES)), **kw)
    res = LAST_RESULT.results
    outb = np.concatenate(
        [np.asarray(res[c]["out"]) for c in range(N_CORES)], axis=0
    )
    return np.ascontiguousarray(outb.astype(np.float32))


# revision 8
# speedup vs baseline: 1.0197x; 1.0197x over previous
"""Masked attention on 8 TRN2 NeuronCores — pure data-parallel over batch.

Full inputs:  q,k,v (16,2048,128) f32, mask (16,2048,2048) bool.
Output:       (16,2048,128) f32.

Per core (2 batches), per 512-q x 128-k score tile:

  scores + mask land in PSUM via TWO fp8 DoubleRow matmuls (each contracts
  2x128 rows at 0.5 cyc/out-row):
    pass1: k_hi . q_hi  +  (-240*I) . mask01     (mask folded into the matmul)
    pass2: k_lo . q_hi  +  k_hi . q_lo           (hi-lo fp8 ~ bf16 precision)
  q/k are split host-side into e4m3 hi/lo pairs; the q_lo*k_lo term is
  dropped (second order).  The two operand pairs of each DoubleRow matmul
  are addressed with strided chunk APs into one big SBUF tile, so no data
  is duplicated.

  exp is split across two engines to break the ACT throughput wall:
    ~5/8 of tiles: ScalarE  attn = exp(scale*psum)            -> bf16
    ~3/8 of tiles: VectorE  attn = bitcast_bf16(int16(A*psum+B))
  (Schraudolph-style exponential: the int16 bits of A*x+B reinterpreted as
  bf16 approximate exp(x*scale) to ~1.5% — measured end-to-end rel err
  1.3e-2 vs the 2e-2 gate.)  Masked entries were pushed down by the -240
  mask term so both paths produce ~0.

  AV accumulates [128q, 129] per q-subblock in bf16 with a ones-column in
  va giving the softmax denominator; normalize = reciprocal + scale on
  DVE/ACT into a staging tile; one DMA per 1024-q group stores the output
  in natural [q, d] layout.
"""

import numpy as np
import ml_dtypes

B, S, D = 16, 2048, 128
N_CORES = 8
BPC = B // N_CORES   # batches per core
P = 128              # partitions
QW = 512             # q-tile width (one PSUM bank of f32)
KB = S // P          # k-blocks per batch (16)
NQT = S // QW        # q-tiles per batch (4)
NQG = NQT // 2       # q-groups: 2 q-tiles per group
NPAIR = KB // 2      # k-block pairs per group pass (8)

SCALE = float(1.0 / np.sqrt(np.float32(128.0)))
A2 = float(128.0 * np.log2(np.e) * SCALE)   # Schraudolph slope
B_U = 16249.0                               # Schraudolph bias (calibrated)
C_MASK = -240.0                             # mask diag coefficient (e4m3 max)
DVE_UNITS = (1, 4, 6)                       # of every 8 (g,t,qh) units

_NC = None
LAST_RESULT = None  # BassKernelResults of the most recent run (for profiling)


def _build_nc(bpc=BPC, s=S):
    import concourse.bacc as bacc
    import concourse.tile as tile
    from concourse import mybir

    BPC_, S_ = bpc, s
    KB_ = S_ // P
    NQT_ = S_ // QW
    NQG_ = NQT_ // 2
    NPAIR_ = KB_ // 2
    PD = 3            # AV matmuls pipelined this many pairs behind exp
    DA = D + 1

    bf16 = mybir.dt.bfloat16
    f32 = mybir.dt.float32
    fp8 = mybir.dt.float8e4
    i16 = mybir.dt.int16
    DR = mybir.MatmulPerfMode.DoubleRow
    MUL = mybir.AluOpType.mult
    ADD = mybir.AluOpType.add

    nc = bacc.Bacc()
    # all [d, s]-transposed fp8 hi/lo halves of q and k
    qhT = nc.declare_dram_parameter("qhT", [BPC_, P, S_], fp8, isOutput=False)
    qlT = nc.declare_dram_parameter("qlT", [BPC_, P, S_], fp8, isOutput=False)
    khT = nc.declare_dram_parameter("khT", [BPC_, P, S_], fp8, isOutput=False)
    klT = nc.declare_dram_parameter("klT", [BPC_, P, S_], fp8, isOutput=False)
    # mask01[b, k, q] = 1.0 where masked else 0.0 (host-transposed)
    m8 = nc.declare_dram_parameter("m8", [BPC_, S_, S_], fp8, isOutput=False)
    # -240 * I
    diag = nc.declare_dram_parameter("diag", [P, P], fp8, isOutput=False)
    # va host-packed as [p, kb*(D+1)]: row p holds v[kb*128+p, :]+[1] per kb
    va = nc.declare_dram_parameter(
        "va", [BPC_, P, KB_ * DA], bf16, isOutput=False
    )
    out = nc.declare_dram_parameter("out", [BPC_, S_, D], bf16, isOutput=True)

    with tile.TileContext(nc) as tc:
        with (
            tc.tile_pool(name="km", bufs=2) as kmp,
            tc.tile_pool(name="qm", bufs=2) as qmp,
            tc.tile_pool(name="vp", bufs=2) as vp,
            tc.tile_pool(name="attn", bufs=10) as attnp,
            tc.tile_pool(name="stg", bufs=3) as stgp,
            tc.tile_pool(name="rp", bufs=8) as rp,
            tc.tile_pool(name="const", bufs=1) as constp,
            tc.tile_pool(name="spsum", bufs=2, space="PSUM") as spsum,
            tc.tile_pool(name="avpsum", bufs=4, space="PSUM") as avpsum,
        ):
            # dummy exp so the activation-table load overlaps initial DMAs
            warm = constp.tile([P, 1], f32)
            nc.vector.memset(warm[:], 0.0)
            nc.scalar.activation(
                warm[:], warm[:], mybir.ActivationFunctionType.Exp
            )
            warm8 = constp.tile([P, P], fp8)
            nc.vector.memset(warm8[:], 0.0)
            # PE warm-up burst: ramps the p-state clock AND fills the
            # otherwise-idle window until the first operand DMAs land
            wp = avpsum.tile([P, P], f32, name="warm_mm", tag="av")
            for _ in range(30):
                nc.tensor.matmul(
                    wp[:], lhsT=warm8[:], rhs=warm8[:], start=True, stop=True
                )
            # deferred-normalize state of the previous q-group
            prev_norm = None  # (av_ps, stage, b, g)
            for b in range(BPC_):
                # KM chunks(128): [0..15]=klT [16..31]=khT [32]=diag
                km_s = kmp.tile([P, 33 * P], fp8)
                # QM chunks(512): [0..31]=masks kb0-7, [32..35]=qhT,
                # [36..39]=qlT, [40..71]=masks kb8-15.  q sits in the middle
                # so every (q-chunk, mask-chunk) stride stays within the
                # 16-bit step_elem ISA field (|step| <= 16384 elements).
                qm_s = qmp.tile([P, 72 * QW], fp8)
                va_s = vp.tile([P, KB_, DA], bf16)

                def _mchunk(kb):
                    return 4 * kb if kb < 8 else 40 + 4 * (kb - 8)

                def _ldm(kb):
                    c0 = _mchunk(kb)
                    nc.sync.dma_start(
                        out=qm_s[:, c0 * QW : (c0 + 4) * QW],
                        in_=m8[b, kb * P : (kb + 1) * P, :],
                    )

                # issue order tuned so early consumers never wait long
                nc.sync.dma_start(
                    out=km_s[:, 32 * P : 33 * P], in_=diag[:, :]
                )
                nc.sync.dma_start(
                    out=km_s[:, 16 * P : 32 * P], in_=khT[b, :, :]
                )
                nc.sync.dma_start(
                    out=qm_s[:, 32 * QW : 36 * QW], in_=qhT[b, :, :]
                )
                _ldm(0)
                _ldm(1)
                nc.sync.dma_start(
                    out=qm_s[:, 36 * QW : 40 * QW], in_=qlT[b, :, :]
                )
                nc.sync.dma_start(out=km_s[:, 0 : 16 * P], in_=klT[b, :, :])
                _ldm(2)
                _ldm(3)
                nc.sync.dma_start(
                    out=va_s[:, :, :],
                    in_=va[b, :, :].rearrange("p (kb d) -> p kb d", d=DA),
                )
                for kb in range(4, KB_):
                    _ldm(kb)

                km3 = km_s[:].rearrange("p (c n) -> p c n", n=P)
                qm3 = qm_s[:].rearrange("p (c n) -> p c n", n=QW)

                def _norm_slot(pav_ps, pstage, sl):
                    # normalize one q-subblock of the previous group
                    recip = rp.tile([P, 1], f32)
                    nc.vector.reciprocal(recip[:], pav_ps[sl][:, D : D + 1])
                    if sl % 2 == 1:
                        nc.scalar.activation(
                            pstage[:, sl, :],
                            pav_ps[sl][:, 0:D],
                            mybir.ActivationFunctionType.Copy,
                            scale=recip[:],
                        )
                    else:
                        nc.vector.tensor_scalar_mul(
                            pstage[:, sl, :], pav_ps[sl][:, 0:D], recip[:]
                        )

                def _store_tri(pstage, pb, pg, tri):
                    s0 = 3 * tri
                    s1 = min(s0 + 3, 8)
                    r0 = pg * 2 * QW + s0 * P
                    out_ap = out[pb, r0 : r0 + (s1 - s0) * P, :].rearrange(
                        "(sl q) d -> q sl d", sl=s1 - s0
                    )
                    nc.sync.dma_start(
                        out=out_ap, in_=pstage[:, s0:s1, :]
                    )

                for g in range(NQG_):
                    last_g = b == BPC_ - 1 and g == NQG_ - 1
                    pd = 1 if last_g else PD
                    av_tri = [
                        avpsum.tile([P, 3, DA], f32, name="av_tri", tag="av")
                        for _ in range(3)
                    ]
                    av_ps = [av_tri[sl // 3][:, sl % 3, :] for sl in range(8)]
                    stage = stgp.tile([P, 8, P], bf16)
                    attn_tiles = [[None, None] for _ in range(NPAIR_)]
                    for t in range(NPAIR_ + pd):
                        if t < NPAIR_:
                            for qh in range(2):
                                qx = g * 2 + qh
                                s_ps = spsum.tile([P, 2, QW], f32)
                                for h in range(2):
                                    kb = 2 * t + h
                                    # pass1: kh.qh + diag.mask
                                    qa = 32 + qx
                                    mc = (
                                        4 * kb if kb < 8 else 40 + 4 * (kb - 8)
                                    ) + qx
                                    sr = mc - qa  # negative for kb<8
                                    stop = mc + (1 if sr > 0 else -1)
                                    if stop < 0:
                                        stop = None
                                    nc.tensor.matmul(
                                        s_ps[:, h, :],
                                        lhsT=km3[
                                            :, 16 + kb : 33 : 16 - kb, :
                                        ],
                                        rhs=qm3[:, qa : stop : sr, :],
                                        start=True,
                                        stop=False,
                                        perf_mode=DR,
                                    )
                                    # pass2: kl.qh + kh.ql
                                    nc.tensor.matmul(
                                        s_ps[:, h, :],
                                        lhsT=km3[:, kb : kb + 17 : 16, :],
                                        rhs=qm3[:, qa : qa + 5 : 4, :],
                                        start=False,
                                        stop=True,
                                        perf_mode=DR,
                                    )
                                unit = (g * NPAIR_ + t) * 2 + qh
                                at = attnp.tile([P, 2, QW], bf16)
                                if (unit % 8) in DVE_UNITS:
                                    nc.vector.tensor_scalar(
                                        at[:, :, :].bitcast(i16),
                                        s_ps[:, :, :],
                                        A2,
                                        B_U,
                                        MUL,
                                        ADD,
                                    )
                                else:
                                    nc.scalar.activation(
                                        at[:, :, :],
                                        s_ps[:, :, :],
                                        mybir.ActivationFunctionType.Exp,
                                        scale=SCALE,
                                    )
                                attn_tiles[t][qh] = at
                        if t >= pd:
                            tp = t - pd
                            for qh in range(2):
                                ats = attn_tiles[tp][qh]
                                for h in range(2):
                                    kb = 2 * tp + h
                                    for qi in range(4):
                                        sl = qh * 4 + qi
                                        nc.tensor.matmul(
                                            av_ps[sl][:, :],
                                            lhsT=ats[
                                                :, h, qi * P : (qi + 1) * P
                                            ],
                                            rhs=va_s[:, kb, :],
                                            start=(kb == 0 and sl % 3 == 0),
                                            stop=(
                                                kb == KB_ - 1
                                                and (sl % 3 == 2 or sl == 7)
                                            ),
                                        )
                        # previous group's normalize, spread one slot per
                        # pair so it never bursts onto the engines right
                        # when this group's exps need them
                        if prev_norm is not None and t < NPAIR_:
                            pav, pstage, pb, pg = prev_norm
                            _norm_slot(pav, pstage, t)
                            if t in (2, 5, 7):
                                _store_tri(pstage, pb, pg, t // 3)
                    prev_norm = (av_ps, stage, b, g)
                    if last_g:
                        for sl in range(8):
                            _norm_slot(av_ps, stage, sl)
                            if sl in (2, 5, 7):
                                _store_tri(stage, b, g, sl // 3)
    nc.compile()
    return nc


def kernel(q, k, v, mask, _trace=False, _trace_kwargs=None):
    global _NC, LAST_RESULT
    from concourse.bass_utils import run_bass_kernel_spmd

    if _NC is None:
        _NC = _build_nc()

    bf = ml_dtypes.bfloat16
    e4 = ml_dtypes.float8_e4m3

    qT = np.ascontiguousarray(np.asarray(q, np.float32).transpose(0, 2, 1))
    kT = np.ascontiguousarray(np.asarray(k, np.float32).transpose(0, 2, 1))
    qh8 = qT.astype(e4)
    ql8 = (qT - qh8.astype(np.float32)).astype(e4)
    kh8 = kT.astype(e4)
    kl8 = (kT - kh8.astype(np.float32)).astype(e4)
    m8_full = np.ascontiguousarray(
        np.asarray(mask, bool).transpose(0, 2, 1)
    ).astype(e4)
    diag = (C_MASK * np.eye(P, dtype=np.float32)).astype(e4)
    ones = np.ones((B, S, 1), dtype=np.float32)
    va_full = (
        np.concatenate([np.asarray(v, np.float32), ones], axis=2)
        .reshape(B, S // P, P, D + 1)
        .transpose(0, 2, 1, 3)
        .reshape(B, P, (S // P) * (D + 1))
        .astype(bf)
    )

    in_maps = []
    for c in range(N_CORES):
        lo, hi = c * BPC, (c + 1) * BPC
        in_maps.append(
            {
                "qhT": qh8[lo:hi],
                "qlT": ql8[lo:hi],
                "khT": kh8[lo:hi],
                "klT": kl8[lo:hi],
                "m8": m8_full[lo:hi],
                "diag": diag,
                "va": va_full[lo:hi],
            }
        )

    kw = {}
    if _trace:
        kw["trace"] = True
        if _trace_kwargs:
            kw.update(_trace_kwargs)
    LAST_RESULT = run_bass_kernel_spmd(_NC, in_maps, list(range(N_CORES)), **kw)
    res = LAST_RESULT.results
    outb = np.concatenate(
        [np.asarray(res[c]["out"]) for c in range(N_CORES)], axis=0
    )
    return np.ascontiguousarray(outb.astype(np.float32))


# revision 14
# speedup vs baseline: 1.0638x; 1.0432x over previous
"""Masked attention on 8 TRN2 NeuronCores — pure data-parallel over batch.

Full inputs:  q,k,v (16,2048,128) f32, mask (16,2048,2048) bool.
Output:       (16,2048,128) f32.

Per core (2 batches), per 512-q x 128-k score tile:

  scores + mask land in PSUM via TWO fp8 DoubleRow matmuls (each contracts
  2x128 rows at 0.5 cyc/out-row):
    pass1: k_hi . q_hi  +  (-240*I) . mask01     (mask folded into the matmul)
    pass2: k_lo . q_hi  +  k_hi . q_lo           (hi-lo fp8 ~ bf16 precision)
  q/k are split host-side into e4m3 hi/lo pairs; the q_lo*k_lo term is
  dropped (second order).  The two operand pairs of each DoubleRow matmul
  are addressed with strided chunk APs into one big SBUF tile, so no data
  is duplicated.

  exp is split across two engines to break the ACT throughput wall:
    ~5/8 of tiles: ScalarE  attn = exp(scale*psum)            -> bf16
    ~3/8 of tiles: VectorE  attn = bitcast_bf16(int16(A*psum+B))
  (Schraudolph-style exponential: the int16 bits of A*x+B reinterpreted as
  bf16 approximate exp(x*scale) to ~1.5% — measured end-to-end rel err
  1.3e-2 vs the 2e-2 gate.)  Masked entries were pushed down by the -240
  mask term so both paths produce ~0.

  AV accumulates [128q, 129] per q-subblock in bf16 with a ones-column in
  va giving the softmax denominator; normalize = reciprocal + scale on
  DVE/ACT into a staging tile; one DMA per 1024-q group stores the output
  in natural [q, d] layout.
"""

import numpy as np
import ml_dtypes

B, S, D = 16, 2048, 128
N_CORES = 8
BPC = B // N_CORES   # batches per core
P = 128              # partitions
QW = 512             # q-tile width (one PSUM bank of f32)
KB = S // P          # k-blocks per batch (16)
NQT = S // QW        # q-tiles per batch (4)
NQG = NQT // 2       # q-groups: 2 q-tiles per group
NPAIR = KB // 2      # k-block pairs per group pass (8)

SCALE = float(1.0 / np.sqrt(np.float32(128.0)))
A2 = float(128.0 * np.log2(np.e) * SCALE)   # Schraudolph slope
B_U = 16249.0                               # Schraudolph bias (calibrated)
C_MASK = -240.0                             # mask diag coefficient (e4m3 max)
# which score half-tiles take the DVE (Schraudolph) path, by halfidx%16
DVE_H16 = (1, 4, 6, 9, 12, 14)

_NC = None
LAST_RESULT = None  # BassKernelResults of the most recent run (for profiling)


def _build_nc(bpc=BPC, s=S):
    import concourse.bacc as bacc
    import concourse.tile as tile
    from concourse import mybir

    BPC_, S_ = bpc, s
    KB_ = S_ // P
    NQT_ = S_ // QW
    NQG_ = NQT_ // 2
    NPAIR_ = KB_ // 2
    PD = 3            # AV matmuls pipelined this many pairs behind exp
    DA = D + 1

    bf16 = mybir.dt.bfloat16
    f32 = mybir.dt.float32
    fp8 = mybir.dt.float8e4
    i16 = mybir.dt.int16
    DR = mybir.MatmulPerfMode.DoubleRow
    MUL = mybir.AluOpType.mult
    ADD = mybir.AluOpType.add

    nc = bacc.Bacc()
    # all [d, s]-transposed fp8 hi/lo halves of q and k
    qhT = nc.declare_dram_parameter("qhT", [BPC_, P, S_], fp8, isOutput=False)
    qlT = nc.declare_dram_parameter("qlT", [BPC_, P, S_], fp8, isOutput=False)
    khT = nc.declare_dram_parameter("khT", [BPC_, P, S_], fp8, isOutput=False)
    klT = nc.declare_dram_parameter("klT", [BPC_, P, S_], fp8, isOutput=False)
    # mask01[b, k, q] = 1.0 where masked else 0.0 (host-transposed)
    m8 = nc.declare_dram_parameter("m8", [BPC_, S_, S_], fp8, isOutput=False)
    # -240 * I
    diag = nc.declare_dram_parameter("diag", [P, P], fp8, isOutput=False)
    # va host-packed as [p, kb*(D+1)]: row p holds v[kb*128+p, :]+[1] per kb
    va = nc.declare_dram_parameter(
        "va", [BPC_, P, KB_ * DA], bf16, isOutput=False
    )
    out = nc.declare_dram_parameter("out", [BPC_, S_, D], bf16, isOutput=True)

    with tile.TileContext(nc) as tc:
        with (
            tc.tile_pool(name="km", bufs=2) as kmp,
            tc.tile_pool(name="qm", bufs=2) as qmp,
            tc.tile_pool(name="vp", bufs=2) as vp,
            tc.tile_pool(name="attn", bufs=10) as attnp,
            tc.tile_pool(name="stg", bufs=3) as stgp,
            tc.tile_pool(name="rp", bufs=8) as rp,
            tc.tile_pool(name="const", bufs=1) as constp,
            tc.tile_pool(name="spsum", bufs=5, space="PSUM") as spsum,
            tc.tile_pool(name="avpsum", bufs=3, space="PSUM") as avpsum,
        ):
            # dummy exp so the activation-table load overlaps initial DMAs
            warm = constp.tile([P, 1], f32)
            nc.vector.memset(warm[:], 0.0)
            nc.scalar.activation(
                warm[:], warm[:], mybir.ActivationFunctionType.Exp
            )
            warm8 = constp.tile([P, P], fp8)
            nc.vector.memset(warm8[:], 0.0)
            # PE warm-up burst: ramps the p-state clock AND fills the
            # otherwise-idle window until the first operand DMAs land
            wp = avpsum.tile([P, P], f32, name="warm_mm", tag="av")
            for _ in range(30):
                nc.tensor.matmul(
                    wp[:], lhsT=warm8[:], rhs=warm8[:], start=True, stop=True
                )
            # deferred-normalize state of the previous q-group
            prev_norm = None  # (av_ps, stage, b, g)
            for b in range(BPC_):
                # KM chunks(128): [0..15]=klT [16..31]=khT [32]=diag
                km_s = kmp.tile([P, 33 * P], fp8)
                # QM chunks(512): [0..31]=masks kb0-7, [32..35]=qhT,
                # [36..39]=qlT, [40..71]=masks kb8-15.  q sits in the middle
                # so every (q-chunk, mask-chunk) stride stays within the
                # 16-bit step_elem ISA field (|step| <= 16384 elements).
                qm_s = qmp.tile([P, 72 * QW], fp8)
                va_s = vp.tile([P, KB_, DA], bf16)

                def _mchunk(kb):
                    return 4 * kb if kb < 8 else 40 + 4 * (kb - 8)

                def _ldm(kb):
                    c0 = _mchunk(kb)
                    nc.sync.dma_start(
                        out=qm_s[:, c0 * QW : (c0 + 4) * QW],
                        in_=m8[b, kb * P : (kb + 1) * P, :],
                    )

                # issue order tuned so early consumers never wait long
                nc.sync.dma_start(
                    out=km_s[:, 32 * P : 33 * P], in_=diag[:, :]
                )
                nc.sync.dma_start(
                    out=km_s[:, 16 * P : 32 * P], in_=khT[b, :, :]
                )
                nc.sync.dma_start(
                    out=qm_s[:, 32 * QW : 36 * QW], in_=qhT[b, :, :]
                )
                _ldm(0)
                _ldm(1)
                nc.sync.dma_start(
                    out=qm_s[:, 36 * QW : 40 * QW], in_=qlT[b, :, :]
                )
                nc.sync.dma_start(out=km_s[:, 0 : 16 * P], in_=klT[b, :, :])
                _ldm(2)
                _ldm(3)
                nc.sync.dma_start(
                    out=va_s[:, :, :],
                    in_=va[b, :, :].rearrange("p (kb d) -> p kb d", d=DA),
                )
                for kb in range(4, KB_):
                    _ldm(kb)

                km3 = km_s[:].rearrange("p (c n) -> p c n", n=P)
                qm3 = qm_s[:].rearrange("p (c n) -> p c n", n=QW)

                def _norm_slot(pav_ps, pstage, sl, act=False):
                    # normalize one q-subblock of the previous group
                    recip = rp.tile([P, 1], f32)
                    nc.vector.reciprocal(recip[:], pav_ps[sl][:, D : D + 1])
                    if act or sl % 2 == 1:
                        nc.scalar.activation(
                            pstage[:, sl, :],
                            pav_ps[sl][:, 0:D],
                            mybir.ActivationFunctionType.Copy,
                            scale=recip[:],
                        )
                    else:
                        nc.vector.tensor_scalar_mul(
                            pstage[:, sl, :], pav_ps[sl][:, 0:D], recip[:]
                        )

                def _store_tri(pstage, pb, pg, tri):
                    s0 = 3 * tri
                    s1 = min(s0 + 3, 8)
                    r0 = pg * 2 * QW + s0 * P
                    out_ap = out[pb, r0 : r0 + (s1 - s0) * P, :].rearrange(
                        "(sl q) d -> q sl d", sl=s1 - s0
                    )
                    nc.sync.dma_start(
                        out=out_ap, in_=pstage[:, s0:s1, :]
                    )

                for g in range(NQG_):
                    last_g = b == BPC_ - 1 and g == NQG_ - 1
                    pd = 1 if last_g else PD
                    av_tri = [
                        avpsum.tile([P, 3, DA], f32, name="av_tri", tag="av")
                        for _ in range(3)
                    ]
                    av_ps = [av_tri[sl // 3][:, sl % 3, :] for sl in range(8)]
                    stage = stgp.tile([P, 8, P], bf16)
                    attn_tiles = [[None, None] for _ in range(NPAIR_)]
                    for t in range(NPAIR_ + pd):
                        if t < NPAIR_:
                            for qh in range(2):
                                qx = g * 2 + qh
                                at = attnp.tile([P, 2, QW], bf16)
                                for h in range(2):
                                    kb = 2 * t + h
                                    s_ps = spsum.tile([P, QW], f32)
                                    # pass1: kh.qh + diag.mask
                                    qa = 32 + qx
                                    mc = (
                                        4 * kb if kb < 8 else 40 + 4 * (kb - 8)
                                    ) + qx
                                    sr = mc - qa  # negative for kb<8
                                    stop = mc + (1 if sr > 0 else -1)
                                    if stop < 0:
                                        stop = None
                                    nc.tensor.matmul(
                                        s_ps[:, :],
                                        lhsT=km3[
                                            :, 16 + kb : 33 : 16 - kb, :
                                        ],
                                        rhs=qm3[:, qa : stop : sr, :],
                                        start=True,
                                        stop=False,
                                        perf_mode=DR,
                                    )
                                    # pass2: kl.qh + kh.ql
                                    nc.tensor.matmul(
                                        s_ps[:, :],
                                        lhsT=km3[:, kb : kb + 17 : 16, :],
                                        rhs=qm3[:, qa : qa + 5 : 4, :],
                                        start=False,
                                        stop=True,
                                        perf_mode=DR,
                                    )
                                    hx = ((g * NPAIR_ + t) * 2 + qh) * 2 + h
                                    if (hx % 16) in DVE_H16:
                                        nc.vector.tensor_scalar(
                                            at[:, h, :].bitcast(i16),
                                            s_ps[:, :],
                                            A2,
                                            B_U,
                                            MUL,
                                            ADD,
                                        )
                                    else:
                                        nc.scalar.activation(
                                            at[:, h, :],
                                            s_ps[:, :],
                                            mybir.ActivationFunctionType.Exp,
                                            scale=SCALE,
                                        )
                                attn_tiles[t][qh] = at
                        # previous group's normalize: two slots per pair over
                        # the first 4 pairs, emitted BEFORE this group's AV
                        # so each av_tri bank is fully drained before the AV
                        # pipeline (starting at t=pd) reuses it
                        if prev_norm is not None and t < 4:
                            pav, pstage, pb, pg = prev_norm
                            _norm_slot(pav, pstage, 2 * t)
                            _norm_slot(pav, pstage, 2 * t + 1)
                            if t in (1, 2, 3):
                                _store_tri(pstage, pb, pg, t - 1)
                        if t >= pd:
                            tp = t - pd
                            for qh in range(2):
                                ats = attn_tiles[tp][qh]
                                for h in range(2):
                                    kb = 2 * tp + h
                                    for qi in range(4):
                                        sl = qh * 4 + qi
                                        nc.tensor.matmul(
                                            av_ps[sl][:, :],
                                            lhsT=ats[
                                                :, h, qi * P : (qi + 1) * P
                                            ],
                                            rhs=va_s[:, kb, :],
                                            start=(kb == 0 and sl % 3 == 0),
                                            stop=(
                                                kb == KB_ - 1
                                                and (sl % 3 == 2 or sl == 7)
                                            ),
                                        )
                    prev_norm = (av_ps, stage, b, g)
                    if last_g:
                        for sl in range(8):
                            _norm_slot(av_ps, stage, sl, act=True)
                            if sl in (2, 5, 7):
                                _store_tri(stage, b, g, sl // 3)
    nc.compile()
    return nc


def kernel(q, k, v, mask, _trace=False, _trace_kwargs=None):
    global _NC, LAST_RESULT
    from concourse.bass_utils import run_bass_kernel_spmd

    if _NC is None:
        _NC = _build_nc()

    bf = ml_dtypes.bfloat16
    e4 = ml_dtypes.float8_e4m3

    qT = np.ascontiguousarray(np.asarray(q, np.float32).transpose(0, 2, 1))
    kT = np.ascontiguousarray(np.asarray(k, np.float32).transpose(0, 2, 1))
    qh8 = qT.astype(e4)
    ql8 = (qT - qh8.astype(np.float32)).astype(e4)
    kh8 = kT.astype(e4)
    kl8 = (kT - kh8.astype(np.float32)).astype(e4)
    m8_full = np.ascontiguousarray(
        np.asarray(mask, bool).transpose(0, 2, 1)
    ).astype(e4)
    diag = (C_MASK * np.eye(P, dtype=np.float32)).astype(e4)
    ones = np.ones((B, S, 1), dtype=np.float32)
    va_full = (
        np.concatenate([np.asarray(v, np.float32), ones], axis=2)
        .reshape(B, S // P, P, D + 1)
        .transpose(0, 2, 1, 3)
        .reshape(B, P, (S // P) * (D + 1))
        .astype(bf)
    )

    in_maps = []
    for c in range(N_CORES):
        lo, hi = c * BPC, (c + 1) * BPC
        in_maps.append(
            {
                "qhT": qh8[lo:hi],
                "qlT": ql8[lo:hi],
                "khT": kh8[lo:hi],
                "klT": kl8[lo:hi],
                "m8": m8_full[lo:hi],
                "diag": diag,
                "va": va_full[lo:hi],
            }
        )

    kw = {}
    if _trace:
        kw["trace"] = True
        if _trace_kwargs:
            kw.update(_trace_kwargs)
    LAST_RESULT = run_bass_kernel_spmd(_NC, in_maps, list(range(N_CORES)), **kw)
    res = LAST_RESULT.results
    outb = np.concatenate(
        [np.asarray(res[c]["out"]) for c in range(N_CORES)], axis=0
    )
    return np.ascontiguousarray(outb.astype(np.float32))


# revision 31
# speedup vs baseline: 1.0921x; 1.0266x over previous
"""Masked attention on 8 TRN2 NeuronCores — pure data-parallel over batch.

Full inputs:  q,k,v (16,2048,128) f32, mask (16,2048,2048) bool.
Output:       (16,2048,128) f32.

Per core (2 batches), per 512-q x 128-k score tile:

  scores + mask land in PSUM via TWO fp8 DoubleRow matmuls (each contracts
  2x128 rows at 0.5 cyc/out-row):
    pass1: k_hi . q_hi  +  (-240*I) . mask01     (mask folded into the matmul)
    pass2: k_lo . q_hi  +  k_hi . q_lo           (hi-lo fp8 ~ bf16 precision)
  q/k are split host-side into e4m3 hi/lo pairs; the q_lo*k_lo term is
  dropped (second order).  The two operand pairs of each DoubleRow matmul
  are addressed with strided chunk APs into one big SBUF tile, so no data
  is duplicated.

  exp is split across two engines to break the ACT throughput wall:
    ~5/8 of tiles: ScalarE  attn = exp(scale*psum)            -> bf16
    ~3/8 of tiles: VectorE  attn = bitcast_bf16(int16(A*psum+B))
  (Schraudolph-style exponential: the int16 bits of A*x+B reinterpreted as
  bf16 approximate exp(x*scale) to ~1.5% — measured end-to-end rel err
  1.3e-2 vs the 2e-2 gate.)  Masked entries were pushed down by the -240
  mask term so both paths produce ~0.

  AV accumulates [128q, 129] per q-subblock in bf16 with a ones-column in
  va giving the softmax denominator; normalize = reciprocal + scale on
  DVE/ACT into a staging tile; one DMA per 1024-q group stores the output
  in natural [q, d] layout.
"""

import numpy as np
import ml_dtypes

B, S, D = 16, 2048, 128
N_CORES = 8
BPC = B // N_CORES   # batches per core
P = 128              # partitions
QW = 512             # q-tile width (one PSUM bank of f32)
KB = S // P          # k-blocks per batch (16)
NQT = S // QW        # q-tiles per batch (4)
NQG = NQT // 2       # q-groups: 2 q-tiles per group
NPAIR = KB // 2      # k-block pairs per group pass (8)

SCALE = float(1.0 / np.sqrt(np.float32(128.0)))
A2 = float(128.0 * np.log2(np.e) * SCALE)   # Schraudolph slope
B_U = 16249.0                               # Schraudolph bias (calibrated)
C_MASK = -240.0                             # mask diag coefficient (e4m3 max)
# which score half-tiles take the DVE (Schraudolph) path, by halfidx%16
DVE_H16 = (1, 3, 6, 9, 11, 12, 14)

_NC = None
LAST_RESULT = None  # BassKernelResults of the most recent run (for profiling)


def _build_nc(bpc=BPC, s=S):
    import concourse.bacc as bacc
    import concourse.tile as tile
    from concourse import mybir

    BPC_, S_ = bpc, s
    KB_ = S_ // P
    NQT_ = S_ // QW
    NQG_ = NQT_ // 2
    NPAIR_ = KB_ // 2
    PD = 3            # AV matmuls pipelined this many pairs behind exp
    DA = D + 1

    bf16 = mybir.dt.bfloat16
    f32 = mybir.dt.float32
    fp8 = mybir.dt.float8e4
    i16 = mybir.dt.int16
    DR = mybir.MatmulPerfMode.DoubleRow
    MUL = mybir.AluOpType.mult
    ADD = mybir.AluOpType.add

    nc = bacc.Bacc()
    # all [d, s]-transposed fp8 hi/lo halves of q and k
    qhT = nc.declare_dram_parameter("qhT", [BPC_, P, S_], fp8, isOutput=False)
    qlT = nc.declare_dram_parameter("qlT", [BPC_, P, S_], fp8, isOutput=False)
    khT = nc.declare_dram_parameter("khT", [BPC_, P, S_], fp8, isOutput=False)
    klT = nc.declare_dram_parameter("klT", [BPC_, P, S_], fp8, isOutput=False)
    # mask01[b, k, q] = 1.0 where masked else 0.0 (host-transposed)
    m8 = nc.declare_dram_parameter("m8", [BPC_, S_, S_], fp8, isOutput=False)
    # -240 * I
    diag = nc.declare_dram_parameter("diag", [P, P], fp8, isOutput=False)
    # va host-packed as [p, kb*(D+1)]: row p holds v[kb*128+p, :]+[1] per kb
    va = nc.declare_dram_parameter(
        "va", [BPC_, P, KB_ * DA], bf16, isOutput=False
    )
    out = nc.declare_dram_parameter("out", [BPC_, S_, D], bf16, isOutput=True)

    with tile.TileContext(nc) as tc:
        with (
            tc.tile_pool(name="km", bufs=2) as kmp,
            tc.tile_pool(name="qm", bufs=2) as qmp,
            tc.tile_pool(name="vp", bufs=2) as vp,
            tc.tile_pool(name="attn", bufs=10) as attnp,
            tc.tile_pool(name="stg", bufs=3) as stgp,
            tc.tile_pool(name="rp", bufs=8) as rp,
            tc.tile_pool(name="const", bufs=1) as constp,
            tc.tile_pool(name="spsum", bufs=5, space="PSUM") as spsum,
            tc.tile_pool(name="avpsum", bufs=3, space="PSUM") as avpsum,
        ):
            # dummy exp so the activation-table load overlaps initial DMAs
            warm = constp.tile([P, 1], f32)
            nc.vector.memset(warm[:], 0.0)
            nc.scalar.activation(
                warm[:], warm[:], mybir.ActivationFunctionType.Exp
            )
            warm8 = constp.tile([P, P], fp8)
            nc.vector.memset(warm8[:], 0.0)
            # PE warm-up burst: ramps the p-state clock AND fills the
            # otherwise-idle window until the first operand DMAs land
            wp = avpsum.tile([P, P], f32, name="warm_mm", tag="av")
            for _ in range(12):
                nc.tensor.matmul(
                    wp[:], lhsT=warm8[:], rhs=warm8[:], start=True, stop=True
                )
            # deferred-normalize state of the previous q-group
            prev_norm = None  # (av_ps, stage, b, g)
            for b in range(BPC_):
                # KM chunks(128): [0..15]=klT [16..31]=khT [32]=diag
                km_s = kmp.tile([P, 33 * P], fp8)
                # QM chunks(512), laid out so the strided rhs AP's bounding
                # range (dependency tracking is range-based) for group-0
                # matmuls only covers mask pieces that load before them:
                #   [0..15]  = q-cols 1024:2048 of masks kb7..kb0 (reversed)
                #   [16..31] = q-cols 0:1024    of masks kb7..kb0 (reversed)
                #   [32..35] = qhT   [36..39] = qlT
                #   [40..55] = q-cols 0:1024    of masks kb8..kb15
                #   [56..71] = q-cols 1024:2048 of masks kb8..kb15
                # All (q-chunk, mask-chunk) strides stay within the 16-bit
                # step_elem ISA field.
                qm_s = qmp.tile([P, 72 * QW], fp8)
                va_s = vp.tile([P, KB_, DA], bf16)

                def _mchunk(kb, qx):
                    if kb < 8:
                        if qx < 2:
                            return 16 + 2 * (7 - kb) + qx
                        return 2 * (7 - kb) + (qx - 2)
                    if qx < 2:
                        return 40 + 2 * (kb - 8) + qx
                    return 56 + 2 * (kb - 8) + (qx - 2)

                def _ldm(kb, half):
                    # one q-column half ([0:1024) or [1024:2048)) of mask kb
                    c0 = _mchunk(kb, 0 if half == 0 else 2)
                    nc.sync.dma_start(
                        out=qm_s[:, c0 * QW : (c0 + 2) * QW],
                        in_=m8[
                            b,
                            kb * P : (kb + 1) * P,
                            2 * half * QW : (2 * half + 2) * QW,
                        ],
                    )

                # issue order tuned so early consumers never wait long;
                # on the first batch the leading operands are split so the
                # first fused matmul's deps land in ~2us of serial DMA
                nc.sync.dma_start(
                    out=km_s[:, 32 * P : 33 * P], in_=diag[:, :]
                )
                nc.sync.dma_start(
                    out=km_s[:, 16 * P : 32 * P], in_=khT[b, :, :]
                )
                nc.sync.dma_start(
                    out=qm_s[:, 32 * QW : 36 * QW], in_=qhT[b, :, :]
                )
                _ldm(0, 0)
                nc.sync.dma_start(
                    out=qm_s[:, 36 * QW : 38 * QW], in_=qlT[b, :, 0 : 2 * QW]
                )
                nc.sync.dma_start(out=km_s[:, 0 : 16 * P], in_=klT[b, :, :])
                _ldm(1, 0)
                nc.sync.dma_start(
                    out=qm_s[:, 38 * QW : 40 * QW], in_=qlT[b, :, 2 * QW :]
                )
                for kb in range(2, 6):
                    _ldm(kb, 0)
                nc.sync.dma_start(
                    out=va_s[:, :, :],
                    in_=va[b, :, :].rearrange("p (kb d) -> p kb d", d=DA),
                )
                for kb in range(6, KB_):
                    _ldm(kb, 0)
                for kb in range(KB_):
                    _ldm(kb, 1)

                km3 = km_s[:].rearrange("p (c n) -> p c n", n=P)
                qm3 = qm_s[:].rearrange("p (c n) -> p c n", n=QW)

                def _norm_slot(pav_ps, pstage, sl, act=False):
                    # normalize one q-subblock of the previous group
                    recip = rp.tile([P, 1], f32)
                    nc.vector.reciprocal(recip[:], pav_ps[sl][:, D : D + 1])
                    if act or sl % 2 == 1:
                        nc.scalar.activation(
                            pstage[:, sl, :],
                            pav_ps[sl][:, 0:D],
                            mybir.ActivationFunctionType.Copy,
                            scale=recip[:],
                        )
                    else:
                        nc.vector.tensor_scalar_mul(
                            pstage[:, sl, :], pav_ps[sl][:, 0:D], recip[:]
                        )

                def _store_tri(pstage, pb, pg, tri):
                    s0 = 3 * tri
                    s1 = min(s0 + 3, 8)
                    r0 = pg * 2 * QW + s0 * P
                    out_ap = out[pb, r0 : r0 + (s1 - s0) * P, :].rearrange(
                        "(sl q) d -> q sl d", sl=s1 - s0
                    )
                    nc.sync.dma_start(
                        out=out_ap, in_=pstage[:, s0:s1, :]
                    )

                for g in range(NQG_):
                    last_g = b == BPC_ - 1 and g == NQG_ - 1
                    pd = 1 if last_g else PD
                    av_tri = [
                        avpsum.tile([P, 3, DA], f32, name="av_tri", tag="av")
                        for _ in range(3)
                    ]
                    av_ps = [av_tri[sl // 3][:, sl % 3, :] for sl in range(8)]
                    stage = stgp.tile([P, 8, P], bf16)
                    attn_tiles = [[None, None] for _ in range(NPAIR_)]
                    for t in range(NPAIR_ + pd):
                        if t < NPAIR_:
                            for qh in range(2):
                                qx = g * 2 + qh
                                at = attnp.tile([P, 2, QW], bf16)
                                for h in range(2):
                                    kb = 2 * t + h
                                    s_ps = spsum.tile([P, QW], f32)
                                    # pass1: kh.qh + diag.mask
                                    qa = 32 + qx
                                    mc = _mchunk(kb, qx)
                                    sr = mc - qa  # negative for kb<8
                                    stop = mc + (1 if sr > 0 else -1)
                                    if stop < 0:
                                        stop = None
                                    nc.tensor.matmul(
                                        s_ps[:, :],
                                        lhsT=km3[
                                            :, 16 + kb : 33 : 16 - kb, :
                                        ],
                                        rhs=qm3[:, qa : stop : sr, :],
                                        start=True,
                                        stop=False,
                                        perf_mode=DR,
                                    )
                                    # pass2: kl.qh + kh.ql
                                    nc.tensor.matmul(
                                        s_ps[:, :],
                                        lhsT=km3[:, kb : kb + 17 : 16, :],
                                        rhs=qm3[:, qa : qa + 5 : 4, :],
                                        start=False,
                                        stop=True,
                                        perf_mode=DR,
                                    )
                                    hx = ((g * NPAIR_ + t) * 2 + qh) * 2 + h
                                    if (hx % 16) in DVE_H16:
                                        nc.vector.tensor_scalar(
                                            at[:, h, :].bitcast(i16),
                                            s_ps[:, :],
                                            A2,
                                            B_U,
                                            MUL,
                                            ADD,
                                        )
                                    else:
                                        nc.scalar.activation(
                                            at[:, h, :],
                                            s_ps[:, :],
                                            mybir.ActivationFunctionType.Exp,
                                            scale=SCALE,
                                        )
                                attn_tiles[t][qh] = at
                        # previous group's normalize: two slots per pair over
                        # the first 4 pairs, emitted BEFORE this group's AV
                        # so each av_tri bank is fully drained before the AV
                        # pipeline (starting at t=pd) reuses it
                        if prev_norm is not None and t < 4:
                            pav, pstage, pb, pg = prev_norm
                            _norm_slot(pav, pstage, 2 * t)
                            _norm_slot(pav, pstage, 2 * t + 1)
                            if t in (1, 2, 3):
                                _store_tri(pstage, pb, pg, t - 1)
                        if t >= pd:
                            tp = t - pd
                            for qh in range(2):
                                ats = attn_tiles[tp][qh]
                                for h in range(2):
                                    kb = 2 * tp + h
                                    for qi in range(4):
                                        sl = qh * 4 + qi
                                        nc.tensor.matmul(
                                            av_ps[sl][:, :],
                                            lhsT=ats[
                                                :, h, qi * P : (qi + 1) * P
                                            ],
                                            rhs=va_s[:, kb, :],
                                            start=(kb == 0 and sl % 3 == 0),
                                            stop=(
                                                kb == KB_ - 1
                                                and (sl % 3 == 2 or sl == 7)
                                            ),
                                        )
                    prev_norm = (av_ps, stage, b, g)
                    if last_g:
                        for sl in range(8):
                            _norm_slot(av_ps, stage, sl)
                            if sl in (2, 5, 7):
                                _store_tri(stage, b, g, sl // 3)
    nc.compile()
    return nc


def kernel(q, k, v, mask, _trace=False, _trace_kwargs=None):
    global _NC, LAST_RESULT
    from concourse.bass_utils import run_bass_kernel_spmd

    if _NC is None:
        _NC = _build_nc()

    bf = ml_dtypes.bfloat16
    e4 = ml_dtypes.float8_e4m3

    qT = np.ascontiguousarray(np.asarray(q, np.float32).transpose(0, 2, 1))
    kT = np.ascontiguousarray(np.asarray(k, np.float32).transpose(0, 2, 1))
    qh8 = qT.astype(e4)
    ql8 = (qT - qh8.astype(np.float32)).astype(e4)
    kh8 = kT.astype(e4)
    kl8 = (kT - kh8.astype(np.float32)).astype(e4)
    m8_full = np.ascontiguousarray(
        np.asarray(mask, bool).transpose(0, 2, 1)
    ).astype(e4)
    diag = (C_MASK * np.eye(P, dtype=np.float32)).astype(e4)
    ones = np.ones((B, S, 1), dtype=np.float32)
    va_full = (
        np.concatenate([np.asarray(v, np.float32), ones], axis=2)
        .reshape(B, S // P, P, D + 1)
        .transpose(0, 2, 1, 3)
        .reshape(B, P, (S // P) * (D + 1))
        .astype(bf)
    )

    in_maps = []
    for c in range(N_CORES):
        lo, hi = c * BPC, (c + 1) * BPC
        in_maps.append(
            {
                "qhT": qh8[lo:hi],
                "qlT": ql8[lo:hi],
                "khT": kh8[lo:hi],
                "klT": kl8[lo:hi],
                "m8": m8_full[lo:hi],
                "diag": diag,
                "va": va_full[lo:hi],
            }
        )

    kw = {}
    if _trace:
        kw["trace"] = True
        if _trace_kwargs:
            kw.update(_trace_kwargs)
    LAST_RESULT = run_bass_kernel_spmd(_NC, in_maps, list(range(N_CORES)), **kw)
    res = LAST_RESULT.results
    outb = np.concatenate(
        [np.asarray(res[c]["out"]) for c in range(N_CORES)], axis=0
    )
    return np.ascontiguousarray(outb.astype(np.float32))


# revision 36
# speedup vs baseline: 1.0925x; 1.0004x over previous
"""Masked attention on 8 TRN2 NeuronCores — pure data-parallel over batch.

Full inputs:  q,k,v (16,2048,128) f32, mask (16,2048,2048) bool.
Output:       (16,2048,128) f32.

Per core (2 batches), per 512-q x 128-k score tile:

  scores + mask land in PSUM via TWO fp8 DoubleRow matmuls (each contracts
  2x128 rows at 0.5 cyc/out-row):
    pass1: k_hi . q_hi  +  (-240*I) . mask01     (mask folded into the matmul)
    pass2: k_lo . q_hi  +  k_hi . q_lo           (hi-lo fp8 ~ bf16 precision)
  q/k are split host-side into e4m3 hi/lo pairs; the q_lo*k_lo term is
  dropped (second order).  The two operand pairs of each DoubleRow matmul
  are addressed with strided chunk APs into one big SBUF tile, so no data
  is duplicated.

  exp is split across two engines to break the ACT throughput wall:
    ~5/8 of tiles: ScalarE  attn = exp(scale*psum)            -> bf16
    ~3/8 of tiles: VectorE  attn = bitcast_bf16(int16(A*psum+B))
  (Schraudolph-style exponential: the int16 bits of A*x+B reinterpreted as
  bf16 approximate exp(x*scale) to ~1.5% — measured end-to-end rel err
  1.3e-2 vs the 2e-2 gate.)  Masked entries were pushed down by the -240
  mask term so both paths produce ~0.

  AV accumulates [128q, 129] per q-subblock in bf16 with a ones-column in
  va giving the softmax denominator; normalize = reciprocal + scale on
  DVE/ACT into a staging tile; one DMA per 1024-q group stores the output
  in natural [q, d] layout.
"""

import numpy as np
import ml_dtypes

B, S, D = 16, 2048, 128
N_CORES = 8
BPC = B // N_CORES   # batches per core
P = 128              # partitions
QW = 512             # q-tile width (one PSUM bank of f32)
KB = S // P          # k-blocks per batch (16)
NQT = S // QW        # q-tiles per batch (4)
NQG = NQT // 2       # q-groups: 2 q-tiles per group
NPAIR = KB // 2      # k-block pairs per group pass (8)

SCALE = float(1.0 / np.sqrt(np.float32(128.0)))
A2 = float(128.0 * np.log2(np.e) * SCALE)   # Schraudolph slope
B_U = 16249.0                               # Schraudolph bias (calibrated)
C_MASK = -240.0                             # mask diag coefficient (e4m3 max)
# which score half-tiles take the DVE (Schraudolph) path, by halfidx%16
DVE_H16 = (1, 3, 6, 9, 11, 12, 14)

_NC = None
LAST_RESULT = None  # BassKernelResults of the most recent run (for profiling)


def _build_nc(bpc=BPC, s=S):
    import concourse.bacc as bacc
    import concourse.tile as tile
    from concourse import mybir

    BPC_, S_ = bpc, s
    KB_ = S_ // P
    NQT_ = S_ // QW
    NQG_ = NQT_ // 2
    NPAIR_ = KB_ // 2
    PD = 3            # AV matmuls pipelined this many pairs behind exp
    DA = D + 1

    bf16 = mybir.dt.bfloat16
    f32 = mybir.dt.float32
    fp8 = mybir.dt.float8e4
    i16 = mybir.dt.int16
    DR = mybir.MatmulPerfMode.DoubleRow
    MUL = mybir.AluOpType.mult
    ADD = mybir.AluOpType.add

    nc = bacc.Bacc()
    # all [d, s]-transposed fp8 hi/lo halves of q and k
    qhT = nc.declare_dram_parameter("qhT", [BPC_, P, S_], fp8, isOutput=False)
    qlT = nc.declare_dram_parameter("qlT", [BPC_, P, S_], fp8, isOutput=False)
    khT = nc.declare_dram_parameter("khT", [BPC_, P, S_], fp8, isOutput=False)
    klT = nc.declare_dram_parameter("klT", [BPC_, P, S_], fp8, isOutput=False)
    # mask01[b, k, q] = 1.0 where masked else 0.0 (host-transposed)
    m8 = nc.declare_dram_parameter("m8", [BPC_, S_, S_], fp8, isOutput=False)
    # -240 * I
    diag = nc.declare_dram_parameter("diag", [P, P], fp8, isOutput=False)
    # va host-packed as [p, kb*(D+1)]: row p holds v[kb*128+p, :]+[1] per kb
    va = nc.declare_dram_parameter(
        "va", [BPC_, P, KB_ * DA], bf16, isOutput=False
    )
    out = nc.declare_dram_parameter("out", [BPC_, S_, D], bf16, isOutput=True)

    with tile.TileContext(nc) as tc:
        with (
            tc.tile_pool(name="km", bufs=2) as kmp,
            tc.tile_pool(name="qm", bufs=2) as qmp,
            tc.tile_pool(name="vp", bufs=2) as vp,
            tc.tile_pool(name="attn", bufs=10) as attnp,
            tc.tile_pool(name="stg", bufs=3) as stgp,
            tc.tile_pool(name="rp", bufs=8) as rp,
            tc.tile_pool(name="const", bufs=1) as constp,
            tc.tile_pool(name="spsum", bufs=5, space="PSUM") as spsum,
            tc.tile_pool(name="avpsum", bufs=3, space="PSUM") as avpsum,
        ):
            # dummy exp so the activation-table load overlaps initial DMAs
            warm = constp.tile([P, 1], f32)
            nc.vector.memset(warm[:], 0.0)
            nc.scalar.activation(
                warm[:], warm[:], mybir.ActivationFunctionType.Exp
            )
            warm8 = constp.tile([P, P], fp8)
            nc.vector.memset(warm8[:], 0.0)
            # PE warm-up burst: ramps the p-state clock AND fills the
            # otherwise-idle window until the first operand DMAs land
            wp = avpsum.tile([P, P], f32, name="warm_mm", tag="av")
            for _ in range(12):
                nc.tensor.matmul(
                    wp[:], lhsT=warm8[:], rhs=warm8[:], start=True, stop=True
                )
            # deferred-normalize state of the previous q-group
            prev_norm = None  # (av_ps, stage, b, g)
            for b in range(BPC_):
                # KM chunks(128): [0..15]=klT [16..31]=khT [32]=diag
                km_s = kmp.tile([P, 33 * P], fp8)
                # QM chunks(512), laid out so the strided rhs AP's bounding
                # range (dependency tracking is range-based) for group-0
                # matmuls only covers mask pieces that load before them:
                #   [0..15]  = q-cols 1024:2048 of masks kb7..kb0 (reversed)
                #   [16..31] = q-cols 0:1024    of masks kb7..kb0 (reversed)
                #   [32..35] = qhT   [36..39] = qlT
                #   [40..55] = q-cols 0:1024    of masks kb8..kb15
                #   [56..71] = q-cols 1024:2048 of masks kb8..kb15
                # All (q-chunk, mask-chunk) strides stay within the 16-bit
                # step_elem ISA field.
                qm_s = qmp.tile([P, 72 * QW], fp8)
                va_s = vp.tile([P, KB_, DA], bf16)

                def _mchunk(kb, qx):
                    if kb < 8:
                        if qx < 2:
                            return 16 + 2 * (7 - kb) + qx
                        return 2 * (7 - kb) + (qx - 2)
                    if qx < 2:
                        return 40 + 2 * (kb - 8) + qx
                    return 56 + 2 * (kb - 8) + (qx - 2)

                def _ldmp(t2, half):
                    # one q-column half of BOTH k-blocks of pair t2 in one
                    # DMA (the low region is laid out reversed, so the two
                    # blocks' chunks are adjacent either way)
                    kb = 2 * t2
                    q0 = 2 * half * QW
                    src = m8[
                        b, kb * P : (kb + 2) * P, q0 : q0 + 2 * QW
                    ].rearrange("(j p) q -> p j q", j=2)
                    if kb < 8:
                        c0 = _mchunk(kb + 1, 0 if half == 0 else 2)
                        src = src[:, ::-1, :]
                    else:
                        c0 = _mchunk(kb, 0 if half == 0 else 2)
                    nc.sync.dma_start(
                        out=qm_s[:, c0 * QW : (c0 + 4) * QW].rearrange(
                            "p (j q) -> p j q", q=2 * QW
                        ),
                        in_=src,
                    )

                # issue order tuned so early consumers never wait long;
                # on the first batch the leading operands are split so the
                # first fused matmul's deps land in ~2us of serial DMA
                nc.sync.dma_start(
                    out=km_s[:, 32 * P : 33 * P], in_=diag[:, :]
                )
                nc.sync.dma_start(
                    out=km_s[:, 16 * P : 32 * P], in_=khT[b, :, :]
                )
                nc.sync.dma_start(
                    out=qm_s[:, 32 * QW : 34 * QW], in_=qhT[b, :, 0 : 2 * QW]
                )
                _ldmp(0, 0)
                nc.sync.dma_start(
                    out=qm_s[:, 36 * QW : 38 * QW], in_=qlT[b, :, 0 : 2 * QW]
                )
                nc.sync.dma_start(out=km_s[:, 0 : 16 * P], in_=klT[b, :, :])
                for t2 in range(1, 6):
                    _ldmp(t2, 0)
                nc.sync.dma_start(
                    out=va_s[:, :, :],
                    in_=va[b, :, :].rearrange("p (kb d) -> p kb d", d=DA),
                )
                for t2 in range(6, NPAIR_):
                    _ldmp(t2, 0)
                nc.sync.dma_start(
                    out=qm_s[:, 34 * QW : 36 * QW], in_=qhT[b, :, 2 * QW :]
                )
                nc.sync.dma_start(
                    out=qm_s[:, 38 * QW : 40 * QW], in_=qlT[b, :, 2 * QW :]
                )
                for t2 in range(NPAIR_):
                    _ldmp(t2, 1)

                km3 = km_s[:].rearrange("p (c n) -> p c n", n=P)
                qm3 = qm_s[:].rearrange("p (c n) -> p c n", n=QW)

                def _norm_slot(pav_ps, pstage, sl, act=None):
                    # normalize one q-subblock of the previous group
                    recip = rp.tile([P, 1], f32)
                    nc.vector.reciprocal(recip[:], pav_ps[sl][:, D : D + 1])
                    if act if act is not None else sl % 2 == 1:
                        nc.scalar.activation(
                            pstage[:, sl, :],
                            pav_ps[sl][:, 0:D],
                            mybir.ActivationFunctionType.Copy,
                            scale=recip[:],
                        )
                    else:
                        nc.vector.tensor_scalar_mul(
                            pstage[:, sl, :], pav_ps[sl][:, 0:D], recip[:]
                        )

                def _store_tri(pstage, pb, pg, tri):
                    s0 = 3 * tri
                    s1 = min(s0 + 3, 8)
                    r0 = pg * 2 * QW + s0 * P
                    out_ap = out[pb, r0 : r0 + (s1 - s0) * P, :].rearrange(
                        "(sl q) d -> q sl d", sl=s1 - s0
                    )
                    nc.sync.dma_start(
                        out=out_ap, in_=pstage[:, s0:s1, :]
                    )

                for g in range(NQG_):
                    last_g = b == BPC_ - 1 and g == NQG_ - 1
                    pd = 1 if last_g else PD
                    av_tri = [
                        avpsum.tile([P, 3, DA], f32, name="av_tri", tag="av")
                        for _ in range(3)
                    ]
                    av_ps = [av_tri[sl // 3][:, sl % 3, :] for sl in range(8)]
                    stage = stgp.tile([P, 8, P], bf16)
                    attn_tiles = [[None, None] for _ in range(NPAIR_)]
                    for t in range(NPAIR_ + pd):
                        if t < NPAIR_:
                            for qh in range(2):
                                qx = g * 2 + qh
                                at = attnp.tile([P, 2, QW], bf16)
                                for h in range(2):
                                    kb = 2 * t + h
                                    s_ps = spsum.tile([P, QW], f32)
                                    # pass1: kh.qh + diag.mask
                                    qa = 32 + qx
                                    mc = _mchunk(kb, qx)
                                    sr = mc - qa  # negative for kb<8
                                    stop = mc + (1 if sr > 0 else -1)
                                    if stop < 0:
                                        stop = None
                                    nc.tensor.matmul(
                                        s_ps[:, :],
                                        lhsT=km3[
                                            :, 16 + kb : 33 : 16 - kb, :
                                        ],
                                        rhs=qm3[:, qa : stop : sr, :],
                                        start=True,
                                        stop=False,
                                        perf_mode=DR,
                                    )
                                    # pass2: kl.qh + kh.ql
                                    nc.tensor.matmul(
                                        s_ps[:, :],
                                        lhsT=km3[:, kb : kb + 17 : 16, :],
                                        rhs=qm3[:, qa : qa + 5 : 4, :],
                                        start=False,
                                        stop=True,
                                        perf_mode=DR,
                                    )
                                    hx = ((g * NPAIR_ + t) * 2 + qh) * 2 + h
                                    if (hx % 16) in DVE_H16:
                                        nc.vector.tensor_scalar(
                                            at[:, h, :].bitcast(i16),
                                            s_ps[:, :],
                                            A2,
                                            B_U,
                                            MUL,
                                            ADD,
                                        )
                                    else:
                                        nc.scalar.activation(
                                            at[:, h, :],
                                            s_ps[:, :],
                                            mybir.ActivationFunctionType.Exp,
                                            scale=SCALE,
                                        )
                                attn_tiles[t][qh] = at
                        # previous group's normalize: two slots per pair over
                        # the first 4 pairs, emitted BEFORE this group's AV
                        # so each av_tri bank is fully drained before the AV
                        # pipeline (starting at t=pd) reuses it
                        if prev_norm is not None and t < 4:
                            pav, pstage, pb, pg = prev_norm
                            _norm_slot(pav, pstage, 2 * t)
                            _norm_slot(pav, pstage, 2 * t + 1)
                            if t in (1, 2, 3):
                                _store_tri(pstage, pb, pg, t - 1)
                        if t >= pd:
                            tp = t - pd
                            for qh in range(2):
                                ats = attn_tiles[tp][qh]
                                for h in range(2):
                                    kb = 2 * tp + h
                                    for qi in range(4):
                                        sl = qh * 4 + qi
                                        nc.tensor.matmul(
                                            av_ps[sl][:, :],
                                            lhsT=ats[
                                                :, h, qi * P : (qi + 1) * P
                                            ],
                                            rhs=va_s[:, kb, :],
                                            start=(kb == 0 and sl % 3 == 0),
                                            stop=(
                                                kb == KB_ - 1
                                                and (sl % 3 == 2 or sl == 7)
                                            ),
                                        )
                    prev_norm = (av_ps, stage, b, g)
                    if last_g:
                        # tail: keep the serial DVE chain (2 exps + 8 recips
                        # + k scales) balanced against ACT's (2 exps + (8-k)
                        # scales) — k=2 minimizes the longer of the two
                        for sl in range(8):
                            _norm_slot(av_ps, stage, sl, act=sl not in (0, 4))
                            if sl in (2, 5, 7):
                                _store_tri(stage, b, g, sl // 3)
    nc.compile()
    return nc


def kernel(q, k, v, mask, _trace=False, _trace_kwargs=None):
    global _NC, LAST_RESULT
    from concourse.bass_utils import run_bass_kernel_spmd

    if _NC is None:
        _NC = _build_nc()

    bf = ml_dtypes.bfloat16
    e4 = ml_dtypes.float8_e4m3

    qT = np.ascontiguousarray(np.asarray(q, np.float32).transpose(0, 2, 1))
    kT = np.ascontiguousarray(np.asarray(k, np.float32).transpose(0, 2, 1))
    qh8 = qT.astype(e4)
    ql8 = (qT - qh8.astype(np.float32)).astype(e4)
    kh8 = kT.astype(e4)
    kl8 = (kT - kh8.astype(np.float32)).astype(e4)
    m8_full = np.ascontiguousarray(
        np.asarray(mask, bool).transpose(0, 2, 1)
    ).astype(e4)
    diag = (C_MASK * np.eye(P, dtype=np.float32)).astype(e4)
    ones = np.ones((B, S, 1), dtype=np.float32)
    va_full = (
        np.concatenate([np.asarray(v, np.float32), ones], axis=2)
        .reshape(B, S // P, P, D + 1)
        .transpose(0, 2, 1, 3)
        .reshape(B, P, (S // P) * (D + 1))
        .astype(bf)
    )

    in_maps = []
    for c in range(N_CORES):
        lo, hi = c * BPC, (c + 1) * BPC
        in_maps.append(
            {
                "qhT": qh8[lo:hi],
                "qlT": ql8[lo:hi],
                "khT": kh8[lo:hi],
                "klT": kl8[lo:hi],
                "m8": m8_full[lo:hi],
                "diag": diag,
                "va": va_full[lo:hi],
            }
        )

    kw = {}
    if _trace:
        kw["trace"] = True
        if _trace_kwargs:
            kw.update(_trace_kwargs)
    LAST_RESULT = run_bass_kernel_spmd(_NC, in_maps, list(range(N_CORES)), **kw)
    res = LAST_RESULT.results
    outb = np.concatenate(
        [np.asarray(res[c]["out"]) for c in range(N_CORES)], axis=0
    )
    return np.ascontiguousarray(outb.astype(np.float32))


# revision 44
# speedup vs baseline: 1.1070x; 1.0133x over previous
"""Masked attention on 8 TRN2 NeuronCores — pure data-parallel over batch.

Full inputs:  q,k,v (16,2048,128) f32, mask (16,2048,2048) bool.
Output:       (16,2048,128) f32.

Per core (2 batches), per 512-q x 128-k score tile:

  scores + mask land in PSUM via TWO fp8 DoubleRow matmuls (each contracts
  2x128 rows at 0.5 cyc/out-row):
    pass1: k_hi . q_hi  +  (-240*I) . mask01     (mask folded into the matmul)
    pass2: k_lo . q_hi  +  k_hi . q_lo           (hi-lo fp8 ~ bf16 precision)
  q/k are split host-side into e4m3 hi/lo pairs; the q_lo*k_lo term is
  dropped (second order).  The two operand pairs of each DoubleRow matmul
  are addressed with strided chunk APs into one big SBUF tile, so no data
  is duplicated.

  exp is split across two engines to break the ACT throughput wall:
    ~5/8 of tiles: ScalarE  attn = exp(scale*psum)            -> bf16
    ~3/8 of tiles: VectorE  attn = bitcast_bf16(int16(A*psum+B))
  (Schraudolph-style exponential: the int16 bits of A*x+B reinterpreted as
  bf16 approximate exp(x*scale) to ~1.5% — measured end-to-end rel err
  1.3e-2 vs the 2e-2 gate.)  Masked entries were pushed down by the -240
  mask term so both paths produce ~0.

  AV accumulates [128q, 129] per q-subblock in bf16 with a ones-column in
  va giving the softmax denominator; normalize = reciprocal + scale on
  DVE/ACT into a staging tile; one DMA per 1024-q group stores the output
  in natural [q, d] layout.
"""

import numpy as np
import ml_dtypes

B, S, D = 16, 2048, 128
N_CORES = 8
BPC = B // N_CORES   # batches per core
P = 128              # partitions
QW = 512             # q-tile width (one PSUM bank of f32)
KB = S // P          # k-blocks per batch (16)
NQT = S // QW        # q-tiles per batch (4)
NQG = NQT // 2       # q-groups: 2 q-tiles per group
NPAIR = KB // 2      # k-block pairs per group pass (8)

SCALE = float(1.0 / np.sqrt(np.float32(128.0)))
A2 = float(128.0 * np.log2(np.e) * SCALE)   # Schraudolph slope
B_U = 16249.0                               # Schraudolph bias (calibrated)
C_MASK = -240.0                             # mask diag coefficient (e4m3 max)
# which score half-tiles take the DVE (Schraudolph) path, by halfidx%32
DVE_H32 = (1, 3, 6, 9, 11, 12, 14, 17, 19, 20, 22, 25, 27, 28, 30)

_NC = None
LAST_RESULT = None  # BassKernelResults of the most recent run (for profiling)


def _build_nc(bpc=BPC, s=S):
    import concourse.bacc as bacc
    import concourse.tile as tile
    from concourse import mybir

    BPC_, S_ = bpc, s
    KB_ = S_ // P
    NQT_ = S_ // QW
    NQG_ = NQT_ // 2
    NPAIR_ = KB_ // 2
    PD = 4            # AV matmuls pipelined this many pairs behind exp
    DA = D + 1

    bf16 = mybir.dt.bfloat16
    f32 = mybir.dt.float32
    fp8 = mybir.dt.float8e4
    i16 = mybir.dt.int16
    DR = mybir.MatmulPerfMode.DoubleRow
    MUL = mybir.AluOpType.mult
    ADD = mybir.AluOpType.add

    nc = bacc.Bacc()
    # all [d, s]-transposed fp8 hi/lo halves of q and k
    qhT = nc.declare_dram_parameter("qhT", [BPC_, P, S_], fp8, isOutput=False)
    qlT = nc.declare_dram_parameter("qlT", [BPC_, P, S_], fp8, isOutput=False)
    khT = nc.declare_dram_parameter("khT", [BPC_, P, S_], fp8, isOutput=False)
    klT = nc.declare_dram_parameter("klT", [BPC_, P, S_], fp8, isOutput=False)
    # mask01[b, k, q] = 1.0 where masked else 0.0 (host-transposed)
    m8 = nc.declare_dram_parameter("m8", [BPC_, S_, S_], fp8, isOutput=False)
    # -240 * I
    diag = nc.declare_dram_parameter("diag", [P, P], fp8, isOutput=False)
    # va host-packed as [p, kb*(D+1)]: row p holds v[kb*128+p, :]+[1] per kb
    va = nc.declare_dram_parameter(
        "va", [BPC_, P, KB_ * DA], bf16, isOutput=False
    )
    out = nc.declare_dram_parameter("out", [BPC_, S_, D], bf16, isOutput=True)

    with tile.TileContext(nc) as tc:
        with (
            tc.tile_pool(name="km", bufs=2) as kmp,
            tc.tile_pool(name="qm", bufs=2) as qmp,
            tc.tile_pool(name="vp", bufs=2) as vp,
            tc.tile_pool(name="attn", bufs=12) as attnp,
            tc.tile_pool(name="stg", bufs=3) as stgp,
            tc.tile_pool(name="rp", bufs=8) as rp,
            tc.tile_pool(name="const", bufs=1) as constp,
            tc.tile_pool(name="spsum", bufs=5, space="PSUM") as spsum,
            tc.tile_pool(name="avpsum", bufs=3, space="PSUM") as avpsum,
        ):
            # dummy exp so the activation-table load overlaps initial DMAs
            warm = constp.tile([P, 1], f32)
            nc.vector.memset(warm[:], 0.0)
            nc.scalar.activation(
                warm[:], warm[:], mybir.ActivationFunctionType.Exp
            )
            warm8 = constp.tile([P, P], fp8)
            nc.vector.memset(warm8[:], 0.0)
            # PE warm-up burst: ramps the p-state clock AND fills the
            # otherwise-idle window until the first operand DMAs land
            wp = avpsum.tile([P, P], f32, name="warm_mm", tag="av")
            for _ in range(12):
                nc.tensor.matmul(
                    wp[:], lhsT=warm8[:], rhs=warm8[:], start=True, stop=True
                )
            # deferred-normalize state of the previous q-group
            prev_norm = None  # (av_ps, stage, b, g)
            for b in range(BPC_):
                # KM chunks(128): [0..15]=klT [16..31]=khT [32]=diag
                km_s = kmp.tile([P, 33 * P], fp8)
                # QM chunks(512), laid out so the strided rhs AP's bounding
                # range (dependency tracking is range-based) for group-0
                # matmuls only covers mask pieces that load before them:
                #   [0..15]  = q-cols 1024:2048 of masks kb7..kb0 (reversed)
                #   [16..31] = q-cols 0:1024    of masks kb7..kb0 (reversed)
                #   [32..35] = qhT   [36..39] = qlT
                #   [40..55] = q-cols 0:1024    of masks kb8..kb15
                #   [56..71] = q-cols 1024:2048 of masks kb8..kb15
                # All (q-chunk, mask-chunk) strides stay within the 16-bit
                # step_elem ISA field.
                qm_s = qmp.tile([P, 72 * QW], fp8)
                va_s = vp.tile([P, KB_, DA], bf16)

                def _mchunk(kb, qx):
                    if kb < 8:
                        if qx < 2:
                            return 16 + 2 * (7 - kb) + qx
                        return 2 * (7 - kb) + (qx - 2)
                    if qx < 2:
                        return 40 + 2 * (kb - 8) + qx
                    return 56 + 2 * (kb - 8) + (qx - 2)

                def _ldmp(t2, half):
                    # one q-column half of BOTH k-blocks of pair t2 in one
                    # DMA (the low region is laid out reversed, so the two
                    # blocks' chunks are adjacent either way)
                    kb = 2 * t2
                    q0 = 2 * half * QW
                    src = m8[
                        b, kb * P : (kb + 2) * P, q0 : q0 + 2 * QW
                    ].rearrange("(j p) q -> p j q", j=2)
                    if kb < 8:
                        c0 = _mchunk(kb + 1, 0 if half == 0 else 2)
                        src = src[:, ::-1, :]
                    else:
                        c0 = _mchunk(kb, 0 if half == 0 else 2)
                    nc.sync.dma_start(
                        out=qm_s[:, c0 * QW : (c0 + 4) * QW].rearrange(
                            "p (j q) -> p j q", q=2 * QW
                        ),
                        in_=src,
                    )

                # issue order tuned so early consumers never wait long;
                # on the first batch the leading operands are split so the
                # first fused matmul's deps land in ~2us of serial DMA
                nc.sync.dma_start(
                    out=km_s[:, 32 * P : 33 * P], in_=diag[:, :]
                )
                nc.sync.dma_start(
                    out=km_s[:, 16 * P : 32 * P], in_=khT[b, :, :]
                )
                nc.sync.dma_start(
                    out=qm_s[:, 32 * QW : 34 * QW], in_=qhT[b, :, 0 : 2 * QW]
                )
                _ldmp(0, 0)
                nc.sync.dma_start(
                    out=qm_s[:, 36 * QW : 38 * QW], in_=qlT[b, :, 0 : 2 * QW]
                )
                nc.sync.dma_start(out=km_s[:, 0 : 16 * P], in_=klT[b, :, :])
                for t2 in range(1, 4):
                    _ldmp(t2, 0)
                nc.sync.dma_start(
                    out=va_s[:, :, :],
                    in_=va[b, :, :].rearrange("p (kb d) -> p kb d", d=DA),
                )
                for t2 in range(4, NPAIR_):
                    _ldmp(t2, 0)
                nc.sync.dma_start(
                    out=qm_s[:, 34 * QW : 36 * QW], in_=qhT[b, :, 2 * QW :]
                )
                nc.sync.dma_start(
                    out=qm_s[:, 38 * QW : 40 * QW], in_=qlT[b, :, 2 * QW :]
                )
                for t2 in range(NPAIR_):
                    _ldmp(t2, 1)

                km3 = km_s[:].rearrange("p (c n) -> p c n", n=P)
                qm3 = qm_s[:].rearrange("p (c n) -> p c n", n=QW)

                def _norm_slot(pav_ps, pstage, sl, act=None):
                    # normalize one q-subblock of the previous group
                    recip = rp.tile([P, 1], f32)
                    nc.vector.reciprocal(recip[:], pav_ps[sl][:, D : D + 1])
                    if act if act is not None else sl % 2 == 1:
                        nc.scalar.activation(
                            pstage[:, sl, :],
                            pav_ps[sl][:, 0:D],
                            mybir.ActivationFunctionType.Copy,
                            scale=recip[:],
                        )
                    else:
                        nc.vector.tensor_scalar_mul(
                            pstage[:, sl, :], pav_ps[sl][:, 0:D], recip[:]
                        )

                def _store_tri(pstage, pb, pg, tri):
                    s0 = 3 * tri
                    s1 = min(s0 + 3, 8)
                    r0 = pg * 2 * QW + s0 * P
                    out_ap = out[pb, r0 : r0 + (s1 - s0) * P, :].rearrange(
                        "(sl q) d -> q sl d", sl=s1 - s0
                    )
                    nc.sync.dma_start(
                        out=out_ap, in_=pstage[:, s0:s1, :]
                    )

                for g in range(NQG_):
                    last_g = b == BPC_ - 1 and g == NQG_ - 1
                    pd = 1 if last_g else PD
                    av_tri = [
                        avpsum.tile([P, 3, DA], f32, name="av_tri", tag="av")
                        for _ in range(3)
                    ]
                    av_ps = [av_tri[sl // 3][:, sl % 3, :] for sl in range(8)]
                    stage = stgp.tile([P, 8, P], bf16)
                    attn_tiles = [[None, None] for _ in range(NPAIR_)]
                    for t in range(NPAIR_ + pd):
                        if t < NPAIR_:
                            for qh in range(2):
                                qx = g * 2 + qh
                                at = attnp.tile([P, 2, QW], bf16)
                                for h in range(2):
                                    kb = 2 * t + h
                                    s_ps = spsum.tile([P, QW], f32)
                                    # pass1: kh.qh + diag.mask
                                    qa = 32 + qx
                                    mc = _mchunk(kb, qx)
                                    sr = mc - qa  # negative for kb<8
                                    stop = mc + (1 if sr > 0 else -1)
                                    if stop < 0:
                                        stop = None
                                    nc.tensor.matmul(
                                        s_ps[:, :],
                                        lhsT=km3[
                                            :, 16 + kb : 33 : 16 - kb, :
                                        ],
                                        rhs=qm3[:, qa : stop : sr, :],
                                        start=True,
                                        stop=False,
                                        perf_mode=DR,
                                    )
                                    # pass2: kl.qh + kh.ql
                                    nc.tensor.matmul(
                                        s_ps[:, :],
                                        lhsT=km3[:, kb : kb + 17 : 16, :],
                                        rhs=qm3[:, qa : qa + 5 : 4, :],
                                        start=False,
                                        stop=True,
                                        perf_mode=DR,
                                    )
                                    hx = ((g * NPAIR_ + t) * 2 + qh) * 2 + h
                                    if (hx % 32) in DVE_H32:
                                        nc.vector.tensor_scalar(
                                            at[:, h, :].bitcast(i16),
                                            s_ps[:, :],
                                            A2,
                                            B_U,
                                            MUL,
                                            ADD,
                                        )
                                    else:
                                        nc.scalar.activation(
                                            at[:, h, :],
                                            s_ps[:, :],
                                            mybir.ActivationFunctionType.Exp,
                                            scale=SCALE,
                                        )
                                attn_tiles[t][qh] = at
                        # previous group's normalize, spread over the first
                        # pairs and emitted BEFORE this group's AV so each
                        # av_tri bank is fully drained before the AV pipeline
                        # (starting at t=pd) reuses it
                        if prev_norm is not None:
                            if pd >= 4:
                                plan = ((0, 1), (2, 3), (4, 5), (6,), (7,))
                                trist = {1: 0, 2: 1, 4: 2}
                            else:
                                plan = ((0, 1), (2, 3), (4, 5), (6, 7))
                                trist = {1: 0, 2: 1, 3: 2}
                            if t < len(plan):
                                pav, pstage, pb, pg = prev_norm
                                for psl in plan[t]:
                                    _norm_slot(pav, pstage, psl)
                                if t in trist:
                                    _store_tri(pstage, pb, pg, trist[t])
                        if t >= pd:
                            tp = t - pd
                            for qh in range(2):
                                ats = attn_tiles[tp][qh]
                                for h in range(2):
                                    kb = 2 * tp + h
                                    for qi in range(4):
                                        sl = qh * 4 + qi
                                        nc.tensor.matmul(
                                            av_ps[sl][:, :],
                                            lhsT=ats[
                                                :, h, qi * P : (qi + 1) * P
                                            ],
                                            rhs=va_s[:, kb, :],
                                            start=(kb == 0 and sl % 3 == 0),
                                            stop=(
                                                kb == KB_ - 1
                                                and (sl % 3 == 2 or sl == 7)
                                            ),
                                        )
                    prev_norm = (av_ps, stage, b, g)
                    if last_g:
                        for sl in range(8):
                            _norm_slot(av_ps, stage, sl)
                            if sl in (2, 5, 7):
                                _store_tri(stage, b, g, sl // 3)
    nc.compile()
    return nc


def kernel(q, k, v, mask, _trace=False, _trace_kwargs=None):
    global _NC, LAST_RESULT
    from concourse.bass_utils import run_bass_kernel_spmd

    if _NC is None:
        _NC = _build_nc()

    bf = ml_dtypes.bfloat16
    e4 = ml_dtypes.float8_e4m3

    qT = np.ascontiguousarray(np.asarray(q, np.float32).transpose(0, 2, 1))
    kT = np.ascontiguousarray(np.asarray(k, np.float32).transpose(0, 2, 1))
    qh8 = qT.astype(e4)
    ql8 = (qT - qh8.astype(np.float32)).astype(e4)
    kh8 = kT.astype(e4)
    kl8 = (kT - kh8.astype(np.float32)).astype(e4)
    m8_full = np.ascontiguousarray(
        np.asarray(mask, bool).transpose(0, 2, 1)
    ).astype(e4)
    diag = (C_MASK * np.eye(P, dtype=np.float32)).astype(e4)
    ones = np.ones((B, S, 1), dtype=np.float32)
    va_full = (
        np.concatenate([np.asarray(v, np.float32), ones], axis=2)
        .reshape(B, S // P, P, D + 1)
        .transpose(0, 2, 1, 3)
        .reshape(B, P, (S // P) * (D + 1))
        .astype(bf)
    )

    in_maps = []
    for c in range(N_CORES):
        lo, hi = c * BPC, (c + 1) * BPC
        in_maps.append(
            {
                "qhT": qh8[lo:hi],
                "qlT": ql8[lo:hi],
                "khT": kh8[lo:hi],
                "klT": kl8[lo:hi],
                "m8": m8_full[lo:hi],
                "diag": diag,
                "va": va_full[lo:hi],
            }
        )

    kw = {}
    if _trace:
        kw["trace"] = True
        if _trace_kwargs:
            kw.update(_trace_kwargs)
    LAST_RESULT = run_bass_kernel_spmd(_NC, in_maps, list(range(N_CORES)), **kw)
    res = LAST_RESULT.results
    outb = np.concatenate(
        [np.asarray(res[c]["out"]) for c in range(N_CORES)], axis=0
    )
    return np.ascontiguousarray(outb.astype(np.float32))


# revision 47
# speedup vs baseline: 1.1082x; 1.0010x over previous
"""Masked attention on 8 TRN2 NeuronCores — pure data-parallel over batch.

Full inputs:  q,k,v (16,2048,128) f32, mask (16,2048,2048) bool.
Output:       (16,2048,128) f32.

Per core (2 batches), per 512-q x 128-k score tile:

  scores + mask land in PSUM via TWO fp8 DoubleRow matmuls (each contracts
  2x128 rows at 0.5 cyc/out-row):
    pass1: k_hi . q_hi  +  (-240*I) . mask01     (mask folded into the matmul)
    pass2: k_lo . q_hi  +  k_hi . q_lo           (hi-lo fp8 ~ bf16 precision)
  q/k are split host-side into e4m3 hi/lo pairs; the q_lo*k_lo term is
  dropped (second order).  The two operand pairs of each DoubleRow matmul
  are addressed with strided chunk APs into one big SBUF tile, so no data
  is duplicated.

  exp is split across two engines to break the ACT throughput wall:
    ~5/8 of tiles: ScalarE  attn = exp(scale*psum)            -> bf16
    ~3/8 of tiles: VectorE  attn = bitcast_bf16(int16(A*psum+B))
  (Schraudolph-style exponential: the int16 bits of A*x+B reinterpreted as
  bf16 approximate exp(x*scale) to ~1.5% — measured end-to-end rel err
  1.3e-2 vs the 2e-2 gate.)  Masked entries were pushed down by the -240
  mask term so both paths produce ~0.

  AV accumulates [128q, 129] per q-subblock in bf16 with a ones-column in
  va giving the softmax denominator; normalize = reciprocal + scale on
  DVE/ACT into a staging tile; one DMA per 1024-q group stores the output
  in natural [q, d] layout.
"""

import numpy as np
import ml_dtypes

B, S, D = 16, 2048, 128
N_CORES = 8
BPC = B // N_CORES   # batches per core
P = 128              # partitions
QW = 512             # q-tile width (one PSUM bank of f32)
KB = S // P          # k-blocks per batch (16)
NQT = S // QW        # q-tiles per batch (4)
NQG = NQT // 2       # q-groups: 2 q-tiles per group
NPAIR = KB // 2      # k-block pairs per group pass (8)

SCALE = float(1.0 / np.sqrt(np.float32(128.0)))
A2 = float(128.0 * np.log2(np.e) * SCALE)   # Schraudolph slope
B_U = 16249.0                               # Schraudolph bias (calibrated)
C_MASK = -240.0                             # mask diag coefficient (e4m3 max)
# which score half-tiles take the DVE (Schraudolph) path, by halfidx%32
DVE_H32 = (1, 3, 6, 9, 11, 12, 14, 17, 19, 20, 22, 25, 27, 28, 30)

_NC = None
LAST_RESULT = None  # BassKernelResults of the most recent run (for profiling)


def _build_nc(bpc=BPC, s=S):
    import concourse.bacc as bacc
    import concourse.tile as tile
    from concourse import mybir

    BPC_, S_ = bpc, s
    KB_ = S_ // P
    NQT_ = S_ // QW
    NQG_ = NQT_ // 2
    NPAIR_ = KB_ // 2
    PD = 4            # AV matmuls pipelined this many pairs behind exp
    DA = D + 1

    bf16 = mybir.dt.bfloat16
    f32 = mybir.dt.float32
    fp8 = mybir.dt.float8e4
    i16 = mybir.dt.int16
    DR = mybir.MatmulPerfMode.DoubleRow
    MUL = mybir.AluOpType.mult
    ADD = mybir.AluOpType.add

    nc = bacc.Bacc()
    # all [d, s]-transposed fp8 hi/lo halves of q and k
    qhT = nc.declare_dram_parameter("qhT", [BPC_, P, S_], fp8, isOutput=False)
    qlT = nc.declare_dram_parameter("qlT", [BPC_, P, S_], fp8, isOutput=False)
    khT = nc.declare_dram_parameter("khT", [BPC_, P, S_], fp8, isOutput=False)
    klT = nc.declare_dram_parameter("klT", [BPC_, P, S_], fp8, isOutput=False)
    # mask01[b, k, q] = 1.0 where masked else 0.0 (host-transposed)
    m8 = nc.declare_dram_parameter("m8", [BPC_, S_, S_], fp8, isOutput=False)
    # -240 * I
    diag = nc.declare_dram_parameter("diag", [P, P], fp8, isOutput=False)
    # va host-packed as [p, kb*(D+1)]: row p holds v[kb*128+p, :]+[1] per kb
    va = nc.declare_dram_parameter(
        "va", [BPC_, P, KB_ * DA], bf16, isOutput=False
    )
    out = nc.declare_dram_parameter("out", [BPC_, S_, D], bf16, isOutput=True)

    with tile.TileContext(nc) as tc:
        with (
            tc.tile_pool(name="km", bufs=2) as kmp,
            tc.tile_pool(name="qm", bufs=2) as qmp,
            tc.tile_pool(name="vp", bufs=2) as vp,
            tc.tile_pool(name="attn", bufs=12) as attnp,
            tc.tile_pool(name="stg", bufs=3) as stgp,
            tc.tile_pool(name="rp", bufs=8) as rp,
            tc.tile_pool(name="const", bufs=1) as constp,
            tc.tile_pool(name="spsum", bufs=5, space="PSUM") as spsum,
            tc.tile_pool(name="avpsum", bufs=3, space="PSUM") as avpsum,
        ):
            # dummy exp so the activation-table load overlaps initial DMAs
            warm = constp.tile([P, 1], f32)
            nc.vector.memset(warm[:], 0.0)
            nc.scalar.activation(
                warm[:], warm[:], mybir.ActivationFunctionType.Exp
            )
            warm8 = constp.tile([P, P], fp8)
            nc.vector.memset(warm8[:], 0.0)
            # PE warm-up burst: ramps the p-state clock AND fills the
            # otherwise-idle window until the first operand DMAs land
            wp = avpsum.tile([P, P], f32, name="warm_mm", tag="av")
            for _ in range(12):
                nc.tensor.matmul(
                    wp[:], lhsT=warm8[:], rhs=warm8[:], start=True, stop=True
                )
            # deferred-normalize state of the previous q-group
            prev_norm = None  # (av_ps, stage, b, g)
            for b in range(BPC_):
                # KM chunks(128): [0..15]=klT [16..31]=khT [32]=diag
                km_s = kmp.tile([P, 33 * P], fp8)
                # QM chunks(512), laid out so the strided rhs AP's bounding
                # range (dependency tracking is range-based) for group-0
                # matmuls only covers mask pieces that load before them:
                #   [0..15]  = q-cols 1024:2048 of masks kb7..kb0 (reversed)
                #   [16..31] = q-cols 0:1024    of masks kb7..kb0 (reversed)
                #   [32..35] = qhT   [36..39] = qlT
                #   [40..55] = q-cols 0:1024    of masks kb8..kb15
                #   [56..71] = q-cols 1024:2048 of masks kb8..kb15
                # All (q-chunk, mask-chunk) strides stay within the 16-bit
                # step_elem ISA field.
                qm_s = qmp.tile([P, 72 * QW], fp8)
                va_s = vp.tile([P, KB_, DA], bf16)

                def _mchunk(kb, qx):
                    if kb < 8:
                        if qx < 2:
                            return 16 + 2 * (7 - kb) + qx
                        return 2 * (7 - kb) + (qx - 2)
                    if qx < 2:
                        return 40 + 2 * (kb - 8) + qx
                    return 56 + 2 * (kb - 8) + (qx - 2)

                def _ldmp(t2, half):
                    # one q-column half of BOTH k-blocks of pair t2 in one
                    # DMA (the low region is laid out reversed, so the two
                    # blocks' chunks are adjacent either way)
                    kb = 2 * t2
                    q0 = 2 * half * QW
                    src = m8[
                        b, kb * P : (kb + 2) * P, q0 : q0 + 2 * QW
                    ].rearrange("(j p) q -> p j q", j=2)
                    if kb < 8:
                        c0 = _mchunk(kb + 1, 0 if half == 0 else 2)
                        src = src[:, ::-1, :]
                    else:
                        c0 = _mchunk(kb, 0 if half == 0 else 2)
                    nc.sync.dma_start(
                        out=qm_s[:, c0 * QW : (c0 + 4) * QW].rearrange(
                            "p (j q) -> p j q", q=2 * QW
                        ),
                        in_=src,
                    )

                # issue order tuned so early consumers never wait long;
                # on the first batch the leading operands are split so the
                # first fused matmul's deps land in ~2us of serial DMA
                nc.sync.dma_start(
                    out=km_s[:, 32 * P : 33 * P], in_=diag[:, :]
                )
                nc.sync.dma_start(
                    out=km_s[:, 16 * P : 32 * P], in_=khT[b, :, :]
                )
                nc.sync.dma_start(
                    out=qm_s[:, 32 * QW : 34 * QW], in_=qhT[b, :, 0 : 2 * QW]
                )
                _ldmp(0, 0)
                nc.sync.dma_start(
                    out=qm_s[:, 36 * QW : 38 * QW], in_=qlT[b, :, 0 : 2 * QW]
                )
                nc.sync.dma_start(out=km_s[:, 0 : 16 * P], in_=klT[b, :, :])
                for t2 in range(1, 4):
                    _ldmp(t2, 0)
                nc.sync.dma_start(
                    out=va_s[:, :, :],
                    in_=va[b, :, :].rearrange("p (kb d) -> p kb d", d=DA),
                )
                for t2 in range(4, NPAIR_):
                    _ldmp(t2, 0)
                nc.sync.dma_start(
                    out=qm_s[:, 34 * QW : 36 * QW], in_=qhT[b, :, 2 * QW :]
                )
                nc.sync.dma_start(
                    out=qm_s[:, 38 * QW : 40 * QW], in_=qlT[b, :, 2 * QW :]
                )
                for t2 in range(NPAIR_):
                    _ldmp(t2, 1)

                km3 = km_s[:].rearrange("p (c n) -> p c n", n=P)
                qm3 = qm_s[:].rearrange("p (c n) -> p c n", n=QW)

                def _norm_slot(pav_ps, pstage, sl, recips, ptri, act=None):
                    # normalize one q-subblock of the previous group; the
                    # reciprocals are computed once per av_tri (strided FD-3
                    # op) the first time a slot of that tri is normalized
                    tri = sl // 3
                    if recips[tri] is None:
                        n = min(3, 8 - 3 * tri)
                        r3 = rp.tile([P, 3], f32)
                        nc.vector.reciprocal(
                            r3[:, 0:n], ptri[tri][:, 0:n, D]
                        )
                        recips[tri] = r3
                    recip = recips[tri][:, sl % 3 : sl % 3 + 1]
                    if act if act is not None else sl % 2 == 1:
                        nc.scalar.activation(
                            pstage[:, sl, :],
                            pav_ps[sl][:, 0:D],
                            mybir.ActivationFunctionType.Copy,
                            scale=recip,
                        )
                    else:
                        nc.vector.tensor_scalar_mul(
                            pstage[:, sl, :], pav_ps[sl][:, 0:D], recip
                        )

                def _store_tri(pstage, pb, pg, tri):
                    s0 = 3 * tri
                    s1 = min(s0 + 3, 8)
                    r0 = pg * 2 * QW + s0 * P
                    out_ap = out[pb, r0 : r0 + (s1 - s0) * P, :].rearrange(
                        "(sl q) d -> q sl d", sl=s1 - s0
                    )
                    nc.sync.dma_start(
                        out=out_ap, in_=pstage[:, s0:s1, :]
                    )

                for g in range(NQG_):
                    last_g = b == BPC_ - 1 and g == NQG_ - 1
                    pd = 1 if last_g else PD
                    av_tri = [
                        avpsum.tile([P, 3, DA], f32, name="av_tri", tag="av")
                        for _ in range(3)
                    ]
                    av_ps = [av_tri[sl // 3][:, sl % 3, :] for sl in range(8)]
                    stage = stgp.tile([P, 8, P], bf16)
                    attn_tiles = [[None, None] for _ in range(NPAIR_)]
                    for t in range(NPAIR_ + pd):
                        if t < NPAIR_:
                            for qh in range(2):
                                qx = g * 2 + qh
                                at = attnp.tile([P, 2, QW], bf16)
                                for h in range(2):
                                    kb = 2 * t + h
                                    s_ps = spsum.tile([P, QW], f32)
                                    # pass1: kh.qh + diag.mask
                                    qa = 32 + qx
                                    mc = _mchunk(kb, qx)
                                    sr = mc - qa  # negative for kb<8
                                    stop = mc + (1 if sr > 0 else -1)
                                    if stop < 0:
                                        stop = None
                                    nc.tensor.matmul(
                                        s_ps[:, :],
                                        lhsT=km3[
                                            :, 16 + kb : 33 : 16 - kb, :
                                        ],
                                        rhs=qm3[:, qa : stop : sr, :],
                                        start=True,
                                        stop=False,
                                        perf_mode=DR,
                                    )
                                    # pass2: kl.qh + kh.ql
                                    nc.tensor.matmul(
                                        s_ps[:, :],
                                        lhsT=km3[:, kb : kb + 17 : 16, :],
                                        rhs=qm3[:, qa : qa + 5 : 4, :],
                                        start=False,
                                        stop=True,
                                        perf_mode=DR,
                                    )
                                    hx = ((g * NPAIR_ + t) * 2 + qh) * 2 + h
                                    if (hx % 32) in DVE_H32:
                                        nc.vector.tensor_scalar(
                                            at[:, h, :].bitcast(i16),
                                            s_ps[:, :],
                                            A2,
                                            B_U,
                                            MUL,
                                            ADD,
                                        )
                                    else:
                                        nc.scalar.activation(
                                            at[:, h, :],
                                            s_ps[:, :],
                                            mybir.ActivationFunctionType.Exp,
                                            scale=SCALE,
                                        )
                                attn_tiles[t][qh] = at
                        # previous group's normalize, spread over the first
                        # pairs and emitted BEFORE this group's AV so each
                        # av_tri bank is fully drained before the AV pipeline
                        # (starting at t=pd) reuses it
                        if prev_norm is not None:
                            if pd >= 4:
                                plan = ((0, 1), (2, 3), (4, 5), (6,), (7,))
                                trist = {1: 0, 2: 1, 4: 2}
                            else:
                                plan = ((0, 1), (2, 3), (4, 5), (6, 7))
                                trist = {1: 0, 2: 1, 3: 2}
                            if t < len(plan):
                                pav, ptri, pstage, pb, pg, prc = prev_norm
                                for psl in plan[t]:
                                    _norm_slot(pav, pstage, psl, prc, ptri)
                                if t in trist:
                                    _store_tri(pstage, pb, pg, trist[t])
                        if t >= pd:
                            tp = t - pd
                            for qh in range(2):
                                ats = attn_tiles[tp][qh]
                                for h in range(2):
                                    kb = 2 * tp + h
                                    for qi in range(4):
                                        sl = qh * 4 + qi
                                        nc.tensor.matmul(
                                            av_ps[sl][:, :],
                                            lhsT=ats[
                                                :, h, qi * P : (qi + 1) * P
                                            ],
                                            rhs=va_s[:, kb, :],
                                            start=(kb == 0 and sl % 3 == 0),
                                            stop=(
                                                kb == KB_ - 1
                                                and (sl % 3 == 2 or sl == 7)
                                            ),
                                        )
                    prev_norm = (av_ps, av_tri, stage, b, g, [None] * 3)
                    if last_g:
                        lrc = [None] * 3
                        for sl in range(8):
                            _norm_slot(av_ps, stage, sl, lrc, av_tri)
                            if sl in (2, 5, 7):
                                _store_tri(stage, b, g, sl // 3)
    nc.compile()
    return nc


def kernel(q, k, v, mask, _trace=False, _trace_kwargs=None):
    global _NC, LAST_RESULT
    from concourse.bass_utils import run_bass_kernel_spmd

    if _NC is None:
        _NC = _build_nc()

    bf = ml_dtypes.bfloat16
    e4 = ml_dtypes.float8_e4m3

    qT = np.ascontiguousarray(np.asarray(q, np.float32).transpose(0, 2, 1))
    kT = np.ascontiguousarray(np.asarray(k, np.float32).transpose(0, 2, 1))
    qh8 = qT.astype(e4)
    ql8 = (qT - qh8.astype(np.float32)).astype(e4)
    kh8 = kT.astype(e4)
    kl8 = (kT - kh8.astype(np.float32)).astype(e4)
    m8_full = np.ascontiguousarray(
        np.asarray(mask, bool).transpose(0, 2, 1)
    ).astype(e4)
    diag = (C_MASK * np.eye(P, dtype=np.float32)).astype(e4)
    ones = np.ones((B, S, 1), dtype=np.float32)
    va_full = (
        np.concatenate([np.asarray(v, np.float32), ones], axis=2)
        .reshape(B, S // P, P, D + 1)
        .transpose(0, 2, 1, 3)
        .reshape(B, P, (S // P) * (D + 1))
        .astype(bf)
    )

    in_maps = []
    for c in range(N_CORES):
        lo, hi = c * BPC, (c + 1) * BPC
        in_maps.append(
            {
                "qhT": qh8[lo:hi],
                "qlT": ql8[lo:hi],
                "khT": kh8[lo:hi],
                "klT": kl8[lo:hi],
                "m8": m8_full[lo:hi],
                "diag": diag,
                "va": va_full[lo:hi],
            }
        )

    kw = {}
    if _trace:
        kw["trace"] = True
        if _trace_kwargs:
            kw.update(_trace_kwargs)
    LAST_RESULT = run_bass_kernel_spmd(_NC, in_maps, list(range(N_CORES)), **kw)
    res = LAST_RESULT.results
    outb = np.concatenate(
        [np.asarray(res[c]["out"]) for c in range(N_CORES)], axis=0
    )
    return np.ascontiguousarray(outb.astype(np.float32))
